# revision 59
# baseline (speedup 1.0000x reference)
"""Trainium2 Bass kernel for nn_FISLayerParameterSharingV2.

Math: dcumsum along an axis with discount d is multiplication by a lower
triangular matrix L[i,j] = d^(i-j).  With H = W = 128 the whole per-(b,t)
chain is expressible as 128x128 matmuls + elementwise products:

    s3  = Ls Z3 Ls^T          (Ls strict lower triangular)
    s2  = Ls (Z2*s3) Ls^T
    out = L  (Z1*s2) L^T      (L inclusive lower triangular)

v2 layout strategy (per core; B sharded 4 per core over 8 cores), for the
uniform-discount case (discount[t] identical for all t, which the reference
setup always produces):

  *  x streamed per (b, 32h-block) as [64c, 32*128] fp16 tiles; einsum
     matmul(lhsT=x[c, w-slice], rhs=alphaT[c, 3T]) emits Z^T tiles
     [w, 3T] per (b,h) which are pivoted into a per-b SBUF buffer
     Bp[w, (k,t,h)] fp16 by Act/DVE copies.
  *  All t share one stationary Ls^T / L^T matrix (uniform d), so the
     big per-t constant tables shrink to single 128x128 tiles and the
     scan multiplier mask to one [128, 1024] tile.
  *  Stages run in transposed space [w, h] on 8-t groups: left Ls
     multiply = 8 PE matmuls into a 2-bank PSUM tile [128, 1024]; right
     Ls^T multiply = one discounted inclusive scan over the 1024-wide
     free dim on DVE (resets at each 128 boundary via the mask), which
     also evacuates PSUM->SBUF fp16.  Elementwise Z*s products run on
     GpSimd/DVE (SBUF only).  Strict shift = AP offset + column memset.
  *  Final stage matmul (lhsT=n1, rhs=L^T) un-transposes to [h, (t,w)];
     Act evacuates PSUM to a fp16 SBUF staging tile which DMAs to a
     DRAM output laid out [b, h, t, w] with 2KB contiguous runs; the
     host transposes to [b, t, h, w] and applies the 2^20 unscale.
  *  fp16 storage everywhere with power-of-2 prescales folded into the
     alphas (the scans amplify ~50x per stage and would overflow fp16).

A non-uniform discount falls back to the per-t v1 module (kept below).
"""

import sys
import numpy as np

for _p in ("/opt/trn_rl_repo",):
    if _p not in sys.path:
        sys.path.insert(0, _p)

B, T, C, H, W = 32, 32, 64, 128, 128
NCORES = 8
BPC = B // NCORES          # batches per core
KA = 3                     # number of alphas
C1, C2, C3 = 2.0 ** -8, 2.0 ** -6, 2.0 ** -6
UNSCALE = 1.0 / (C1 * C2 * C3)

_CACHE = {}


def _build_module_v2():
    """Uniform-discount module: shared stationaries, 8t-merged scans,
    fp16 [b,h,t,w] DRAM output."""
    import concourse.bass as bass
    import concourse.mybir as mybir
    import concourse.tile as tile
    from concourse import bacc
    from contextlib import ExitStack

    dt = mybir.dt
    f32, f16 = dt.float32, dt.float16

    nc = bacc.Bacc(
        "TRN2", target_bir_lowering=False, debug=False, num_devices=NCORES
    )
    xs = nc.declare_dram_parameter("xs", [BPC, C, H, W], f16, isOutput=False)
    alphaT = nc.declare_dram_parameter("alphaT", [C, KA * T], f16, isOutput=False)
    statT = nc.declare_dram_parameter("statT", [128, 128], f16, isOutput=False)
    stat2T = nc.declare_dram_parameter("stat2T", [128, 128], f16, isOutput=False)
    frhsT = nc.declare_dram_parameter("frhsT", [128, 128], f16, isOutput=False)
    dmask = nc.declare_dram_parameter("dmask", [128, 1024], f32, isOutput=False)
    outp = nc.declare_dram_parameter("outp", [BPC, H, T, W], f16, isOutput=True)

    HB = 32                    # h-block size for x streaming
    NHB = H // HB              # 4 h-blocks per b
    TG = 8                     # t-group size per stage tick
    NG = T // TG               # 4 t-groups per b
    NT = BPC * NG              # 16 stage ticks
    JU = 8                     # einsum h-slices per psum unit
    NU = HB // JU              # 2 einsum units per (b, hb)
    KCONV = 16                 # groups >= KCONV use the mm-chain final
    MULT = mybir.AluOpType.mult
    ADD = mybir.AluOpType.add

    with tile.TileContext(nc) as tc, ExitStack() as ctx:
        const_pool = ctx.enter_context(tc.tile_pool(name="const", bufs=1))
        xpool = ctx.enter_context(tc.tile_pool(name="xp", bufs=8))
        bppool = ctx.enter_context(tc.tile_pool(name="bp", bufs=3))
        ypool = ctx.enter_context(tc.tile_pool(name="yp", bufs=4))
        mpool = ctx.enter_context(tc.tile_pool(name="mp", bufs=4))
        n1pool = ctx.enter_context(tc.tile_pool(name="n1", bufs=3))
        stgpool = ctx.enter_context(tc.tile_pool(name="stg", bufs=3))
        pspool = ctx.enter_context(tc.tile_pool(name="ps", bufs=2, space="PSUM"))

        atlw = const_pool.tile([1, 8], f32, tag="atlw")
        nc.scalar.memzero(atlw[:])
        alpha_t = const_pool.tile([C, KA * T], f16, tag="alpha")
        nc.gpsimd.dma_start(alpha_t[:], alphaT[:])
        stat_t = const_pool.tile([128, 128], f16, tag="stat")
        stat2_t = const_pool.tile([128, 128], f16, tag="stat2")
        fr_t = const_pool.tile([128, 128], f16, tag="fr")
        dm_t = const_pool.tile([128, 1024], f32, tag="dm")

        def load_stage_consts():
            nc.gpsimd.dma_start(stat_t[:], statT[:])
            nc.gpsimd.dma_start(stat2_t[:], stat2T[:])
            nc.gpsimd.dma_start(fr_t[:], frhsT[:])
            nc.gpsimd.dma_start(dm_t[:], dmask[:])

        x_tiles = {}

        def load_x(b, hb):
            xt = xpool.tile([C, HB * W], f16, tag="x", name=f"xt{b}_{hb}")
            src = xs[b, :, hb * HB : (hb + 1) * HB, :]
            nc.sync.dma_start(xt[:], src.rearrange("c h w -> c (h w)"))
            x_tiles[(b, hb)] = xt

        bp_tiles = {}
        bpv = {}

        def make_bp(b):
            t_ = bppool.tile(
                [128, KA * T * 128], f16, tag="bp", name=f"bp{b}"
            )
            bp_tiles[b] = t_
            bpv[b] = t_[:].rearrange("p (k t h) -> p k t h", k=KA, t=T)

        def einsum_unit(b, hb, u, evac_eng, tag="pe", ksplit=False):
            """One einsum unit: 8 matmuls [w,96] + one pivot evacuation."""
            xt = x_tiles[(b, hb)]
            pts = pspool.tile([128, JU * KA * T], f32, tag=tag, name="pe")
            for j in range(JU):
                jj = u * JU + j
                nc.tensor.matmul(
                    pts[:, j * 96 : (j + 1) * 96],
                    lhsT=xt[:, jj * W : (jj + 1) * W],
                    rhs=alpha_t[:],
                    skip_group_check=True,
                )
            h0 = hb * HB + u * JU
            src_ap = pts[:].rearrange("p (j k t) -> p j k t", j=JU, k=KA)
            dst_ap = (
                bpv[b][:, :, :, h0 : h0 + JU]
                .rearrange("p k t j -> p j k t")
            )
            if ksplit:
                # z3 pieces stream on DVE (fast, unblocks stage-1 early);
                # z1/z2 pieces stream on Act in parallel.
                nc.vector.tensor_copy(dst_ap[:, :, 2:3], src_ap[:, :, 2:3])
                nc.scalar.copy(dst_ap[:, :, 0:2], src_ap[:, :, 0:2])
            elif evac_eng == "v":
                nc.vector.tensor_copy(dst_ap, src_ap)
            else:
                nc.scalar.copy(dst_ap, src_ap)
            return None

        live = {}

        def s3mms(k):
            b, g = divmod(k, NG)
            v = bpv[b]
            p3 = pspool.tile([128, 1024], f32, tag="st", name="p3")
            for tl in range(TG):
                t_ = g * TG + tl
                nc.tensor.matmul(
                    p3[:, tl * 128 : (tl + 1) * 128],
                    lhsT=stat_t[:],
                    rhs=v[:, 2, t_, :],
                    skip_group_check=True,
                )
            live[(k, "p3")] = p3

        KC1 = 99                   # disabled: trades DVE scan for DVE psum-mul (no net)

        def s3mm_chain(k):
            # w-contraction first, output un-transposed [h, (t,w)] in PSUM,
            # evacuated fp16 by Act (idle in the tail region).
            b, g = divmod(k, NG)
            v = bpv[b]
            q = pspool.tile([128, 1024], f32, tag="st", name="q3")
            for tl in range(TG):
                t_ = g * TG + tl
                nc.tensor.matmul(
                    q[:, tl * 128 : (tl + 1) * 128],
                    lhsT=v[:, 2, t_, :],
                    rhs=stat_t[:],
                    skip_group_check=True,
                )
            e3 = n1pool.tile([128, 1024], f16, tag="n1", name="e3")
            nc.scalar.copy(e3[:], q[:])
            live[(k, "e3")] = e3

        def s3mm2_mul2(k):
            # h-direction strict discounted sum as a matmul with (Ls/d)^T
            # (the /d matches the scan path's shifted-read semantics), then
            # m2 = z2 * s3 with the DVE mul reading PSUM directly.  The
            # strict matrix zeroes h=0, so no column memset is needed.
            b, g = divmod(k, NG)
            v = bpv[b][:, :, g * TG : g * TG + TG, :]
            e3 = live.pop((k, "e3"))
            s3p = pspool.tile([128, 1024], f32, tag="st", name="s3p")
            for tl in range(TG):
                nc.tensor.matmul(
                    s3p[:, tl * 128 : (tl + 1) * 128],
                    lhsT=e3[:, tl * 128 : (tl + 1) * 128],
                    rhs=stat2_t[:],
                    skip_group_check=True,
                )
            m2 = mpool.tile([128, 1024], f16, tag="m", name="m2")
            nc.vector.tensor_mul(
                m2[:].rearrange("p (t h) -> p t h", t=TG),
                v[:, 1, :, :],
                s3p[:].rearrange("p (t h) -> p t h", t=TG),
            )
            live[(k, "m2")] = m2

        def scan3(k):
            p3 = live.pop((k, "p3"))
            y3 = ypool.tile([128, 1025], f16, tag="y", name="y3")
            nc.vector.tensor_tensor_scan(
                y3[:, 1:1025], dm_t[:], p3[:],
                initial=0.0, op0=MULT, op1=ADD,
            )
            live[(k, "y3")] = y3

        def _mul_half(v, ki, m, y, half, eng):
            # m[half] = z_k ⊙ shifted-scan, one 512-wide half (4 t-blocks)
            t04 = slice(half * 4, half * 4 + 4)
            mv = m[:].rearrange("p (t h) -> p t h", t=TG)[:, t04, :]
            eng.tensor_mul(
                mv,
                v[:, ki, t04, :],
                y[:, half * 512 : half * 512 + 512].rearrange(
                    "p (t h) -> p t h", t=4
                ),
            )
            eng.memset(mv[:, :, 0:1], 0.0)

        def mul2(k, half, eng):
            b, g = divmod(k, NG)
            v = bpv[b][:, :, g * TG : g * TG + TG, :]
            if (k, "m2") not in live:
                live[(k, "m2")] = mpool.tile(
                    [128, 1024], f16, tag="m", name="m2"
                )
            _mul_half(v, 1, live[(k, "m2")], live[(k, "y3")], half, eng)
            if half == 1:
                del live[(k, "y3")]

        def s2mms(k):
            m2 = live.pop((k, "m2"))
            p2 = pspool.tile([128, 1024], f32, tag="st", name="p2")
            for tl in range(TG):
                nc.tensor.matmul(
                    p2[:, tl * 128 : (tl + 1) * 128],
                    lhsT=stat_t[:],
                    rhs=m2[:, tl * 128 : (tl + 1) * 128],
                    skip_group_check=True,
                )
            live[(k, "p2")] = p2

        def scan2(k):
            p2 = live.pop((k, "p2"))
            y2 = ypool.tile([128, 1025], f16, tag="y", name="y2")
            nc.vector.tensor_tensor_scan(
                y2[:, 1:1025], dm_t[:], p2[:],
                initial=0.0, op0=MULT, op1=ADD,
            )
            live[(k, "y2")] = y2

        def mul1(k, half, eng):
            b, g = divmod(k, NG)
            v = bpv[b][:, :, g * TG : g * TG + TG, :]
            if (k, "m1") not in live:
                live[(k, "m1")] = mpool.tile(
                    [128, 1024], f16, tag="m", name="m1"
                )
            _mul_half(v, 0, live[(k, "m1")], live[(k, "y2")], half, eng)
            if half == 1:
                del live[(k, "y2")]

        def scan1(k):
            m1 = live.pop((k, "m1"))
            n1 = n1pool.tile([128, 1024], f16, tag="n1", name="n1")
            nc.vector.tensor_tensor_scan(
                n1[:], dm_t[:], m1[:],
                initial=0.0, op0=MULT, op1=ADD,
            )
            live[(k, "n1")] = n1

        def fmm1(k):
            # mm-chain final stage (replaces scan1+fmms for late groups,
            # where Act is idle): q = contraction over w first (output
            # un-transposed [h, (t,w)]), evac to fp16, then the h-direction
            # discounted sum is a matmul with the same L^T constant.
            m1 = live.pop((k, "m1"))
            q = pspool.tile([128, 1024], f32, tag="st", name="q")
            for tl in range(TG):
                nc.tensor.matmul(
                    q[:, tl * 128 : (tl + 1) * 128],
                    lhsT=m1[:, tl * 128 : (tl + 1) * 128],
                    rhs=fr_t[:],
                    skip_group_check=True,
                )
            qe = n1pool.tile([128, 1024], f16, tag="n1", name="qe")
            nc.scalar.copy(qe[:], q[:])
            live[(k, "qe")] = qe

        def fmm2(k):
            qe = live.pop((k, "qe"))
            pf = pspool.tile([128, 1024], f32, tag="st", name="pf")
            for tl in range(TG):
                nc.tensor.matmul(
                    pf[:, tl * 128 : (tl + 1) * 128],
                    lhsT=fr_t[:],
                    rhs=qe[:, tl * 128 : (tl + 1) * 128],
                    skip_group_check=True,
                )
            live[(k, "pf")] = pf

        def fmms(k):
            n1 = live.pop((k, "n1"))
            pf = pspool.tile([128, 1024], f32, tag="st", name="pf")
            for tl in range(TG):
                nc.tensor.matmul(
                    pf[:, tl * 128 : (tl + 1) * 128],
                    lhsT=n1[:, tl * 128 : (tl + 1) * 128],
                    rhs=fr_t[:],
                    skip_group_check=True,
                )
            live[(k, "pf")] = pf

        def evac_out(k):
            b, g = divmod(k, NG)
            t0 = g * TG
            pf = live.pop((k, "pf"))
            stg = stgpool.tile([128, 1024], f16, tag="stg", name="stg")
            nc.scalar.copy(stg[:], pf[:])
            dst = outp[b, :, t0 : t0 + TG, :]
            nc.sync.dma_start(
                dst, stg[:].rearrange("p (t w) -> p t w", t=TG)
            )

        # ---- schedule ----
        # head: constants + x(b0)+x(b1) + einsum(b0)
        for hb in range(NHB):
            load_x(0, hb)
        load_stage_consts()
        for hb in range(NHB):
            load_x(1, hb)
        make_bp(0)
        for hb in range(NHB):
            for u in range(NU):
                n = hb * NU + u
                einsum_unit(0, hb, u, "v" if n % 2 == 0 else "s",
                            tag="st" if n % 2 else "pe", ksplit=True)
        s3mms(0)

        # steady: 16 stage ticks + 3 drain ticks; einsum(b+1) and x
        # prefetch ride inside the ticks.  Emission order within a tick is
        # readiness order (oldest group first) so in-order engines don't
        # stall on newest dependencies.
        for k in range(NT + 3):
            b, g = divmod(min(k, NT - 1), NG)
            nb = b + 1 if k < NT else BPC
            if k < NT and g == 0 and nb < BPC:
                make_bp(nb)
            if nb < BPC:
                for u2 in range(2):
                    uu = g * 4 + u2
                    einsum_unit(nb, uu // NU, uu % NU,
                                "v" if uu >= 14 else "s")
            if 0 <= k - 3 < NT:
                if k - 3 >= KCONV:
                    fmm1(k - 3)
                else:
                    scan1(k - 3)
            if 0 <= k - 2 < NT:
                s2mms(k - 2)
            if 0 <= k - 1 < NT:
                if k - 1 >= KC1:
                    s3mm2_mul2(k - 1)
                else:
                    scan3(k - 1)
                    mul2(k - 1, 0, nc.gpsimd)
                    mul2(k - 1, 1, nc.gpsimd)
            if 0 <= k - 2 < NT:
                scan2(k - 2)
                mul1(k - 2, 0, nc.gpsimd)
            if 0 < k < NT:
                if k >= KC1:
                    s3mm_chain(k)
                else:
                    s3mms(k)
            if 0 <= k - 3 < NT:
                if k - 3 >= KCONV:
                    fmm2(k - 3)
                else:
                    fmms(k - 3)
                evac_out(k - 3)
            if 0 <= k - 2 < NT:
                mul1(k - 2, 1, nc.vector)
            if nb < BPC:
                for u2 in range(2):
                    uu = g * 4 + 2 + u2
                    einsum_unit(nb, uu // NU, uu % NU,
                                "v" if uu >= 14 else "s")
            if k < NT and b + 2 < BPC:
                load_x(b + 2, g)

    nc.compile()
    return nc


def _host_prep_v2(alpha_1, alpha_2, alpha_3, d):
    a1 = alpha_1.T * (C1 * d * d)
    alphaT = np.concatenate(
        [a1, alpha_2.T * C2, alpha_3.T * C3], axis=1
    ).astype(np.float16)                     # [C, 3T]

    idx = np.arange(H)
    E = idx[:, None] - idx[None, :]
    P = d ** np.maximum(E, 0)
    L = np.where(E >= 0, P, 0.0)
    Ls = np.where(E >= 1, P, 0.0)
    statT = Ls.T.astype(np.float16).copy()
    stat2T = (Ls / d).T.astype(np.float16).copy()
    frhsT = L.T.astype(np.float16).copy()
    dmask = np.full((128, 1024), d, np.float32)
    dmask[:, 0::128] = 0.0
    return alphaT, statT, stat2T, frhsT, dmask


def kernel_v2(x, alpha_1, alpha_2, alpha_3, d):
    from concourse.bass_utils import run_bass_kernel_spmd

    alphaT, statT, stat2T, frhsT, dmask = _host_prep_v2(
        alpha_1, alpha_2, alpha_3, d
    )
    x = np.ascontiguousarray(x.astype(np.float16))
    key = ("nc_v2",)
    if key not in _CACHE:
        _CACHE[key] = _build_module_v2()
    nc = _CACHE[key]

    shared = {
        "alphaT": alphaT,
        "statT": statT,
        "stat2T": stat2T,
        "frhsT": frhsT,
        "dmask": dmask,
    }
    in_maps = [
        {"xs": x[i * BPC : (i + 1) * BPC], **shared} for i in range(NCORES)
    ]
    res = run_bass_kernel_spmd(nc, in_maps, core_ids=list(range(NCORES)))
    outs = [res.results[i]["outp"] for i in range(NCORES)]
    full = np.concatenate(outs, axis=0)               # [B, H, T, W] f16
    return (
        full.transpose(0, 2, 1, 3).astype(np.float32) * np.float32(UNSCALE)
    )


# ---------------------------------------------------------------------------
# v1 fallback (non-uniform discounts): per-t stationaries, fp32 output.
# ---------------------------------------------------------------------------

def _build_module(uniform_d=False):
    import concourse.bass as bass
    import concourse.mybir as mybir
    import concourse.tile as tile
    from concourse import bacc
    from contextlib import ExitStack

    dt = mybir.dt
    f32, f16 = dt.float32, dt.float16

    nc = bacc.Bacc(
        "TRN2", target_bir_lowering=False, debug=False, num_devices=NCORES
    )
    xs = nc.declare_dram_parameter("xs", [BPC, C, H, W], f32, isOutput=False)
    alphaT = nc.declare_dram_parameter("alphaT", [128, KA * T], f16, isOutput=False)
    stat3T = nc.declare_dram_parameter("stat3T", [128, T * 128], f16, isOutput=False)
    frhs = nc.declare_dram_parameter("frhs", [128, T * 128], f16, isOutput=False)
    dmask = nc.declare_dram_parameter("dmask", [128, T * 128], f32, isOutput=False)
    out = nc.declare_dram_parameter("out", [BPC, T, H, W], f32, isOutput=True)

    HB = 32                    # h-block size for x streaming
    NHB = H // HB              # 4 h-blocks
    NPAIR = BPC // 2           # 2 b-pairs
    NG = T // 4                # 8 t-quad groups
    KCONV = 16                 # groups >= KCONV use the mm-chain final
    MULT = mybir.AluOpType.mult
    ADD = mybir.AluOpType.add
    COPY = mybir.ActivationFunctionType.Copy

    with tile.TileContext(nc) as tc, ExitStack() as ctx:
        const_pool = ctx.enter_context(tc.tile_pool(name="const", bufs=1))
        xpool = ctx.enter_context(tc.tile_pool(name="xp", bufs=4))
        bppool = ctx.enter_context(tc.tile_pool(name="bp", bufs=4))
        ypool = ctx.enter_context(tc.tile_pool(name="yp", bufs=6))
        mpool = ctx.enter_context(tc.tile_pool(name="mp", bufs=6))
        n1pool = ctx.enter_context(tc.tile_pool(name="n1", bufs=4))
        stpool = ctx.enter_context(tc.tile_pool(name="st", bufs=4))
        pspool = ctx.enter_context(tc.tile_pool(name="ps", bufs=8, space="PSUM"))

        # constants: alpha first (einsum needs it immediately); the big
        # stage constants are DMA'd after x(b0) so they don't delay it.
        alpha_t = const_pool.tile([128, KA * T], f16, tag="alpha")
        nc.sync.dma_start(alpha_t[:], alphaT[:])
        s3_t = const_pool.tile([128, T * 128], f16, tag="s3m")
        fr_t = const_pool.tile([128, T * 128], f16, tag="frm")
        dm_t = const_pool.tile([128, T * 128], f32, tag="dmm")

        def load_stage_consts_head():
            # only the first-group slices early: the bulk must not queue
            # ahead of pair-0's remaining x chunks on the FIFO DMA rings
            nc.sync.dma_start(s3_t[:, 0:512], stat3T[:, 0:512])
            nc.sync.dma_start(dm_t[:, 0:512], dmask[:, 0:512])
            nc.sync.dma_start(fr_t[:, 0:512], frhs[:, 0:512])

        def load_stage_consts_rest():
            nc.sync.dma_start(s3_t[:, 512:], stat3T[:, 512:])
            nc.sync.dma_start(dm_t[:, 512:], dmask[:, 512:])
            nc.sync.dma_start(fr_t[:, 512:], frhs[:, 512:])

        bp_tiles = {}   # pair -> [tile, tile]
        bpv = {}        # pair -> rearranged views

        def make_bp(pair):
            bp_tiles[pair] = [
                bppool.tile(
                    [128, KA * T * 128], f16, tag="bp", name=f"bp{pair}_{i}"
                )
                for i in range(2)
            ]
            bpv[pair] = [
                t[:].rearrange("p (k t h) -> p k t h", k=KA, t=T)
                for t in bp_tiles[pair]
            ]

        def einsum_units(pair):
            """Generator of closures: x-DMA + (mms, pivot-evac) units."""
            for hb in range(NHB):
                holder = {}

                def ensure_x(pair=pair, hb=hb, holder=holder):
                    if "xt" not in holder:
                        xt = xpool.tile([128, HB * W], f16, tag="x", name="xt")
                        src = xs[
                            2 * pair : 2 * pair + 2, :, hb * HB : (hb + 1) * HB, :
                        ]
                        nc.gpsimd.dma_start(
                            xt[:], src.rearrange("b c h w -> (b c) (h w)")
                        )
                        holder["xt"] = xt
                    return holder["xt"]

                joff = 0
                for ui, ng in enumerate((5, 5, 5, 5, 4, 4, 4)):
                    def unit(pair=pair, hb=hb, ng=ng, joff=joff,
                             ensure_x=ensure_x, ui=ui):
                        xt = ensure_x()
                        pts = [
                            pspool.tile(
                                [128, ng * KA * T], f32, tag="ps", name=f"pe{i}"
                            )
                            for i in range(2)
                        ]
                        for j in range(ng):
                            for par in range(2):
                                nc.tensor.matmul(
                                    pts[par][:, j * 96 : (j + 1) * 96],
                                    lhsT=xt[
                                        64 * par : 64 * par + 64,
                                        (joff + j) * W : (joff + j + 1) * W,
                                    ],
                                    rhs=alpha_t[64 * par : 64 * par + 64, :],
                                    tile_position=(64 * par, 0),
                                    skip_group_check=True,
                                )
                        h0 = hb * HB + joff
                        for par in range(2):
                            src_ap = pts[par][:].rearrange(
                                "p (j k t) -> p j k t", j=ng, k=KA
                            )
                            dst_ap = (
                                bp_tiles[pair][par][:]
                                .rearrange("p (k t h) -> p k t h", k=KA, t=T)[
                                    :, :, :, h0 : h0 + ng
                                ]
                                .rearrange("p k t j -> p j k t")
                            )
                            gi = hb * 7 + ui
                            if pair == 0 and par == 0 and gi % 3 != 2:
                                nc.vector.tensor_copy(dst_ap, src_ap)
                            else:
                                nc.scalar.copy(dst_ap, src_ap)
                    yield unit
                    joff += ng

        # per-(pair, par, group) live state for the staged pipeline
        live = {}

        def s3mms(pair, par, g):
            v = bpv[pair][par]
            t0 = 4 * g
            p3 = pspool.tile([128, 512], f32, tag="ps", name="p3")
            for tl in range(4):
                t = t0 + tl
                nc.tensor.matmul(
                    p3[:, tl * 128 : (tl + 1) * 128],
                    lhsT=s3_t[:, t * 128 : (t + 1) * 128],
                    rhs=v[:, 2, t, :],
                    skip_group_check=True,
                )
            live[(pair, par, g, "p3")] = p3

        def scan3_mul2(pair, par, g):
            v = bpv[pair][par]
            t0 = 4 * g
            p3 = live.pop((pair, par, g, "p3"))
            y3 = ypool.tile([128, 516], f16, tag="y", name="y3")
            nc.vector.tensor_tensor_scan(
                y3[:, 1:513], dm_t[:, t0 * 128 : t0 * 128 + 512], p3[:],
                initial=0.0, op0=MULT, op1=ADD,
            )
            m2 = mpool.tile([128, 512], f16, tag="m", name="m2")
            nc.gpsimd.tensor_mul(
                m2[:].rearrange("p (t h) -> p t h", t=4),
                v[:, 1, t0 : t0 + 4, :],
                y3[:, 0:512].rearrange("p (t h) -> p t h", t=4),
            )
            nc.gpsimd.memset(
                m2[:].rearrange("p (t h) -> p t h", t=4)[:, :, 0:1], 0.0
            )
            live[(pair, par, g, "m2")] = m2

        def s2mms_scan2_mul1(pair, par, g):
            v = bpv[pair][par]
            t0 = 4 * g
            m2 = live.pop((pair, par, g, "m2"))
            p2 = pspool.tile([128, 512], f32, tag="ps", name="p2")
            for tl in range(4):
                t = t0 + tl
                nc.tensor.matmul(
                    p2[:, tl * 128 : (tl + 1) * 128],
                    lhsT=s3_t[:, t * 128 : (t + 1) * 128],
                    rhs=m2[:, tl * 128 : (tl + 1) * 128],
                    skip_group_check=True,
                )
            y2 = ypool.tile([128, 516], f16, tag="y", name="y2")
            nc.vector.tensor_tensor_scan(
                y2[:, 1:513], dm_t[:, t0 * 128 : t0 * 128 + 512], p2[:],
                initial=0.0, op0=MULT, op1=ADD,
            )
            m1 = mpool.tile([128, 512], f16, tag="m", name="m1")
            eng = nc.gpsimd if g % 2 == 1 else nc.vector
            eng.tensor_mul(
                m1[:].rearrange("p (t h) -> p t h", t=4),
                v[:, 0, t0 : t0 + 4, :],
                y2[:, 0:512].rearrange("p (t h) -> p t h", t=4),
            )
            eng.memset(
                m1[:].rearrange("p (t h) -> p t h", t=4)[:, :, 0:1], 0.0
            )
            live[(pair, par, g, "m1")] = m1

        def scan1_fmms_evac(pair, par, g):
            b = 2 * pair + par
            t0 = 4 * g
            m1 = live.pop((pair, par, g, "m1"))
            n1 = n1pool.tile([128, 512], f16, tag="n1", name="n1")
            nc.vector.tensor_tensor_scan(
                n1[:], dm_t[:, t0 * 128 : t0 * 128 + 512], m1[:],
                initial=0.0, op0=MULT, op1=ADD,
            )
            pf = pspool.tile([128, 512], f32, tag="ps", name="pf")
            for tl in range(4):
                t = t0 + tl
                nc.tensor.matmul(
                    pf[:, tl * 128 : (tl + 1) * 128],
                    lhsT=n1[:, tl * 128 : (tl + 1) * 128],
                    rhs=fr_t[:, t * 128 : (t + 1) * 128],
                    skip_group_check=True,
                )
            half = g // 2
            if g % 2 == 0:
                live[(pair, par, half, "st")] = stpool.tile(
                    [128, 8 * 128], f32, tag="st", name="stg"
                )
            stg = live[(pair, par, half, "st")]
            nc.scalar.activation(
                stg[:, (g % 2) * 512 : (g % 2 + 1) * 512],
                pf[:], COPY, scale=UNSCALE,
            )
            if g % 2 == 1:
                del live[(pair, par, half, "st")]
                dst = out[b, half * 8 : half * 8 + 8, :, :]
                nc.sync.dma_start(
                    dst.rearrange("t h w -> h t w"),
                    stg[:].rearrange("p (t w) -> p t w", t=8),
                )

        def stage_ticks(pair, extra=None):
            for k in range(NG + 3):
                for par in range(2):
                    if k < NG:
                        s3mms(pair, par, k)
                    if 0 <= k - 1 < NG:
                        scan3_mul2(pair, par, k - 1)
                    if 0 <= k - 2 < NG:
                        s2mms_scan2_mul1(pair, par, k - 2)
                    if 0 <= k - 3 < NG:
                        scan1_fmms_evac(pair, par, k - 3)
                if extra is not None:
                    for _ in range(3):
                        u = next(extra, None)
                        if u is not None:
                            u()

        make_bp(0)
        units0 = list(einsum_units(0))
        units0[0]()
        load_stage_consts_head()
        for u in units0[1:]:
            u()
        load_stage_consts_rest()
        make_bp(1)
        it1 = iter(list(einsum_units(1)))
        stage_ticks(0, extra=it1)
        for u in it1:
            u()
        stage_ticks(1)

    nc.compile()
    return nc


def _host_prep(alpha_1, alpha_2, alpha_3, discount):
    ds = np.asarray(discount, dtype=np.float64).reshape(T)
    # stage-2's (d*Ls) and the final-matmul's (d*L^T) d-factors are folded
    # into the alpha_1 columns (everything downstream of them is linear).
    a1scaled = alpha_1.T * (C1 * ds[None, :] ** 2)
    alphaT = np.concatenate(
        [a1scaled, alpha_2.T * C2, alpha_3.T * C3], axis=1
    ).astype(np.float16)
    alphaT_dup = np.concatenate([alphaT, alphaT], axis=0)  # [128, 96]

    idx = np.arange(H)
    E = idx[:, None] - idx[None, :]
    stat3T = np.zeros((128, T * 128), np.float16)
    frhs = np.zeros((128, T * 128), np.float16)
    dmask = np.zeros((128, T * 128), np.float32)
    for t in range(T):
        d = ds[t]
        P = d ** np.maximum(E, 0)
        L = np.where(E >= 0, P, 0.0)
        Ls = np.where(E >= 1, P, 0.0)
        sl = slice(t * 128, (t + 1) * 128)
        stat3T[:, sl] = Ls.T.astype(np.float16)
        frhs[:, sl] = L.T.astype(np.float16)
        dmask[:, sl] = np.float32(d)
        dmask[:, t * 128] = 0.0
    return alphaT_dup, stat3T, frhs, dmask


def _kernel_v1(x, alpha_1, alpha_2, alpha_3, discount):
    from concourse.bass_utils import run_bass_kernel_spmd

    alphaT_dup, stat3T, frhs, dmask = _host_prep(
        alpha_1, alpha_2, alpha_3, discount
    )
    key = ("nc", False)
    if key not in _CACHE:
        _CACHE[key] = _build_module(uniform_d=False)
    nc = _CACHE[key]

    shared = {
        "alphaT": alphaT_dup,
        "stat3T": stat3T,
        "frhs": frhs,
        "dmask": dmask,
    }
    in_maps = [
        {"xs": x[i * BPC : (i + 1) * BPC], **shared} for i in range(NCORES)
    ]
    res = run_bass_kernel_spmd(nc, in_maps, core_ids=list(range(NCORES)))
    outs = [res.results[i]["out"] for i in range(NCORES)]
    return np.concatenate(outs, axis=0).astype(np.float32)


def kernel(x, alpha_1, alpha_2, alpha_3, discount):
    x = np.ascontiguousarray(np.asarray(x, dtype=np.float32))
    a1 = np.asarray(alpha_1, np.float32)
    a2 = np.asarray(alpha_2, np.float32)
    a3 = np.asarray(alpha_3, np.float32)
    ds = np.asarray(discount, dtype=np.float64).reshape(T)

    if np.all(ds == ds[0]):
        return kernel_v2(x, a1, a2, a3, float(ds[0]))
    return _kernel_v1(x, a1, a2, a3, discount)


if __name__ == "__main__":
    import reference as ref

    inputs = {k: np.asarray(v) for k, v in ref.setup_inputs().items()}
    got = kernel(**inputs)
    print("kernel output shape:", got.shape, got.dtype)


# revision 74
# speedup vs baseline: 1.0170x; 1.0170x over previous
"""Trainium2 Bass kernel for nn_FISLayerParameterSharingV2.

Math: dcumsum along an axis with discount d is multiplication by a lower
triangular matrix L[i,j] = d^(i-j).  With H = W = 128 the whole per-(b,t)
chain is expressible as 128x128 matmuls + elementwise products:

    s3  = Ls Z3 Ls^T          (Ls strict lower triangular)
    s2  = Ls (Z2*s3) Ls^T
    out = L  (Z1*s2) L^T      (L inclusive lower triangular)

v2 layout strategy (per core; B sharded 4 per core over 8 cores), for the
uniform-discount case (discount[t] identical for all t, which the reference
setup always produces):

  *  x streamed per (b, 32h-block) as [64c, 32*128] fp16 tiles; einsum
     matmul(lhsT=x[c, w-slice], rhs=alphaT[c, 3T]) emits Z^T tiles
     [w, 3T] per (b,h) which are pivoted into a per-b SBUF buffer
     Bp[w, (k,t,h)] fp16 by Act/DVE copies.
  *  All t share one stationary Ls^T / L^T matrix (uniform d), so the
     big per-t constant tables shrink to single 128x128 tiles and the
     scan multiplier mask to one [128, 1024] tile.
  *  Stages run in transposed space [w, h] on 8-t groups: left Ls
     multiply = 8 PE matmuls into a 2-bank PSUM tile [128, 1024]; right
     Ls^T multiply = one discounted inclusive scan over the 1024-wide
     free dim on DVE (resets at each 128 boundary via the mask), which
     also evacuates PSUM->SBUF fp16.  Elementwise Z*s products run on
     GpSimd/DVE (SBUF only).  Strict shift = AP offset + column memset.
  *  Final stage matmul (lhsT=n1, rhs=L^T) un-transposes to [h, (t,w)];
     Act evacuates PSUM to a fp16 SBUF staging tile which DMAs to a
     DRAM output laid out [b, h, t, w] with 2KB contiguous runs; the
     host transposes to [b, t, h, w] and applies the 2^20 unscale.
  *  fp16 storage everywhere with power-of-2 prescales folded into the
     alphas (the scans amplify ~50x per stage and would overflow fp16).

A non-uniform discount falls back to the per-t v1 module (kept below).
"""

import sys
import numpy as np

for _p in ("/opt/trn_rl_repo",):
    if _p not in sys.path:
        sys.path.insert(0, _p)

B, T, C, H, W = 32, 32, 64, 128, 128
NCORES = 8
BPC = B // NCORES          # batches per core
KA = 3                     # number of alphas
C1, C2, C3 = 2.0 ** -8, 2.0 ** -6, 2.0 ** -6
UNSCALE = 1.0 / (C1 * C2 * C3)

_CACHE = {}


def _build_module_v2():
    """Uniform-discount module: shared stationaries, 8t-merged scans,
    fp16 [b,h,t,w] DRAM output."""
    import concourse.bass as bass
    import concourse.mybir as mybir
    import concourse.tile as tile
    from concourse import bacc
    from contextlib import ExitStack

    dt = mybir.dt
    f32, f16 = dt.float32, dt.float16

    nc = bacc.Bacc(
        "TRN2", target_bir_lowering=False, debug=False, num_devices=NCORES
    )
    xs = nc.declare_dram_parameter("xs", [BPC, C, H, W], f16, isOutput=False)
    alphaT = nc.declare_dram_parameter("alphaT", [C, KA * T], f16, isOutput=False)
    statT = nc.declare_dram_parameter("statT", [128, 128], f16, isOutput=False)
    stat2T = nc.declare_dram_parameter("stat2T", [128, 128], f16, isOutput=False)
    frhsT = nc.declare_dram_parameter("frhsT", [128, 128], f16, isOutput=False)
    dmask = nc.declare_dram_parameter("dmask", [128, 1024], f32, isOutput=False)
    outp = nc.declare_dram_parameter("outp", [BPC, H, T, W], f16, isOutput=True)

    HB = 32                    # h-block size for x streaming
    NHB = H // HB              # 4 h-blocks per b
    TG = 8                     # t-group size per stage tick
    NG = T // TG               # 4 t-groups per b
    NT = BPC * NG              # 16 stage ticks
    JU = 8                     # einsum h-slices per psum unit
    NU = HB // JU              # 2 einsum units per (b, hb)
    KCONV = 16                 # groups >= KCONV use the mm-chain final
    MULT = mybir.AluOpType.mult
    ADD = mybir.AluOpType.add

    with tile.TileContext(nc) as tc, ExitStack() as ctx:
        const_pool = ctx.enter_context(tc.tile_pool(name="const", bufs=1))
        xpool = ctx.enter_context(tc.tile_pool(name="xp", bufs=8))
        bppool = ctx.enter_context(tc.tile_pool(name="bp", bufs=3))
        ypool = ctx.enter_context(tc.tile_pool(name="yp", bufs=4))
        mpool = ctx.enter_context(tc.tile_pool(name="mp", bufs=4))
        n1pool = ctx.enter_context(tc.tile_pool(name="n1", bufs=3))
        stgpool = ctx.enter_context(tc.tile_pool(name="stg", bufs=3))
        pspool = ctx.enter_context(tc.tile_pool(name="ps", bufs=2, space="PSUM"))

        atlw = const_pool.tile([1, 8], f32, tag="atlw")
        nc.scalar.memzero(atlw[:])
        alpha_t = const_pool.tile([C, KA * T], f16, tag="alpha")
        nc.gpsimd.dma_start(alpha_t[:], alphaT[:])
        stat_t = const_pool.tile([128, 128], f16, tag="stat")
        stat2_t = const_pool.tile([128, 128], f16, tag="stat2")
        fr_t = const_pool.tile([128, 128], f16, tag="fr")
        dm_t = const_pool.tile([128, 1024], f32, tag="dm")

        def load_stage_consts():
            nc.gpsimd.dma_start(stat_t[:], statT[:])
            nc.gpsimd.dma_start(stat2_t[:], stat2T[:])
            nc.gpsimd.dma_start(fr_t[:], frhsT[:])
            nc.gpsimd.dma_start(dm_t[:], dmask[:])

        x_tiles = {}

        def load_x(b, hb):
            xt = xpool.tile([C, HB * W], f16, tag="x", name=f"xt{b}_{hb}")
            src = xs[b, :, hb * HB : (hb + 1) * HB, :]
            nc.sync.dma_start(xt[:], src.rearrange("c h w -> c (h w)"))
            x_tiles[(b, hb)] = xt

        bp_tiles = {}
        bpv = {}

        def make_bp(b):
            t_ = bppool.tile(
                [128, KA * T * 128], f16, tag="bp", name=f"bp{b}"
            )
            bp_tiles[b] = t_
            bpv[b] = t_[:].rearrange("p (k t h) -> p k t h", k=KA, t=T)

        def einsum_unit(b, hb, u, evac_eng, tag="pe", ksplit=False,
                        xt=None, ubase=0):
            """One einsum unit: 8 matmuls [w,96] + one pivot evacuation."""
            if xt is None:
                xt = x_tiles[(b, hb)]
            pts = pspool.tile([128, JU * KA * T], f32, tag=tag, name="pe")
            for j in range(JU):
                jj = (u - ubase) * JU + j
                nc.tensor.matmul(
                    pts[:, j * 96 : (j + 1) * 96],
                    lhsT=xt[:, jj * W : (jj + 1) * W],
                    rhs=alpha_t[:],
                    skip_group_check=True,
                )
            h0 = hb * HB + u * JU
            src_ap = pts[:].rearrange("p (j k t) -> p j k t", j=JU, k=KA)
            dst_ap = (
                bpv[b][:, :, :, h0 : h0 + JU]
                .rearrange("p k t j -> p j k t")
            )
            if ksplit:
                # z3 pieces stream on DVE (fast, unblocks stage-1 early);
                # z1/z2 pieces stream on Act in parallel.
                nc.vector.tensor_copy(dst_ap[:, :, 2:3], src_ap[:, :, 2:3])
                nc.scalar.copy(dst_ap[:, :, 0:2], src_ap[:, :, 0:2])
            elif evac_eng == "v":
                nc.vector.tensor_copy(dst_ap, src_ap)
            else:
                nc.scalar.copy(dst_ap, src_ap)
            return None

        live = {}

        def s3mms(k):
            b, g = divmod(k, NG)
            v = bpv[b]
            p3 = pspool.tile([128, 1024], f32, tag="st", name="p3")
            for tl in range(TG):
                t_ = g * TG + tl
                nc.tensor.matmul(
                    p3[:, tl * 128 : (tl + 1) * 128],
                    lhsT=stat_t[:],
                    rhs=v[:, 2, t_, :],
                    skip_group_check=True,
                )
            live[(k, "p3")] = p3

        KC1 = 99                   # disabled: trades DVE scan for DVE psum-mul (no net)

        def s3mm_chain(k):
            # w-contraction first, output un-transposed [h, (t,w)] in PSUM,
            # evacuated fp16 by Act (idle in the tail region).
            b, g = divmod(k, NG)
            v = bpv[b]
            q = pspool.tile([128, 1024], f32, tag="st", name="q3")
            for tl in range(TG):
                t_ = g * TG + tl
                nc.tensor.matmul(
                    q[:, tl * 128 : (tl + 1) * 128],
                    lhsT=v[:, 2, t_, :],
                    rhs=stat_t[:],
                    skip_group_check=True,
                )
            e3 = n1pool.tile([128, 1024], f16, tag="n1", name="e3")
            nc.scalar.copy(e3[:], q[:])
            live[(k, "e3")] = e3

        def s3mm2_mul2(k):
            # h-direction strict discounted sum as a matmul with (Ls/d)^T
            # (the /d matches the scan path's shifted-read semantics), then
            # m2 = z2 * s3 with the DVE mul reading PSUM directly.  The
            # strict matrix zeroes h=0, so no column memset is needed.
            b, g = divmod(k, NG)
            v = bpv[b][:, :, g * TG : g * TG + TG, :]
            e3 = live.pop((k, "e3"))
            s3p = pspool.tile([128, 1024], f32, tag="st", name="s3p")
            for tl in range(TG):
                nc.tensor.matmul(
                    s3p[:, tl * 128 : (tl + 1) * 128],
                    lhsT=e3[:, tl * 128 : (tl + 1) * 128],
                    rhs=stat2_t[:],
                    skip_group_check=True,
                )
            m2 = mpool.tile([128, 1024], f16, tag="m", name="m2")
            nc.vector.tensor_mul(
                m2[:].rearrange("p (t h) -> p t h", t=TG),
                v[:, 1, :, :],
                s3p[:].rearrange("p (t h) -> p t h", t=TG),
            )
            live[(k, "m2")] = m2

        def scan3(k):
            p3 = live.pop((k, "p3"))
            y3 = ypool.tile([128, 1025], f16, tag="y", name="y3")
            nc.vector.tensor_tensor_scan(
                y3[:, 1:1025], dm_t[:], p3[:],
                initial=0.0, op0=MULT, op1=ADD,
            )
            live[(k, "y3")] = y3

        def _mul_half(v, ki, m, y, half, eng):
            # m[half] = z_k ⊙ shifted-scan, one 512-wide half (4 t-blocks)
            t04 = slice(half * 4, half * 4 + 4)
            mv = m[:].rearrange("p (t h) -> p t h", t=TG)[:, t04, :]
            eng.tensor_mul(
                mv,
                v[:, ki, t04, :],
                y[:, half * 512 : half * 512 + 512].rearrange(
                    "p (t h) -> p t h", t=4
                ),
            )
            eng.memset(mv[:, :, 0:1], 0.0)

        def mul2(k, half, eng):
            b, g = divmod(k, NG)
            v = bpv[b][:, :, g * TG : g * TG + TG, :]
            if (k, "m2") not in live:
                live[(k, "m2")] = mpool.tile(
                    [128, 1024], f16, tag="m", name="m2"
                )
            _mul_half(v, 1, live[(k, "m2")], live[(k, "y3")], half, eng)
            if half == 1:
                del live[(k, "y3")]

        def s2mms(k):
            m2 = live.pop((k, "m2"))
            p2 = pspool.tile([128, 1024], f32, tag="st", name="p2")
            for tl in range(TG):
                nc.tensor.matmul(
                    p2[:, tl * 128 : (tl + 1) * 128],
                    lhsT=stat_t[:],
                    rhs=m2[:, tl * 128 : (tl + 1) * 128],
                    skip_group_check=True,
                )
            live[(k, "p2")] = p2

        def scan2(k):
            p2 = live.pop((k, "p2"))
            y2 = ypool.tile([128, 1025], f16, tag="y", name="y2")
            nc.vector.tensor_tensor_scan(
                y2[:, 1:1025], dm_t[:], p2[:],
                initial=0.0, op0=MULT, op1=ADD,
            )
            live[(k, "y2")] = y2

        def mul1(k, half, eng):
            b, g = divmod(k, NG)
            v = bpv[b][:, :, g * TG : g * TG + TG, :]
            if (k, "m1") not in live:
                live[(k, "m1")] = mpool.tile(
                    [128, 1024], f16, tag="m", name="m1"
                )
            _mul_half(v, 0, live[(k, "m1")], live[(k, "y2")], half, eng)
            if half == 1:
                del live[(k, "y2")]

        def scan1(k):
            m1 = live.pop((k, "m1"))
            n1 = n1pool.tile([128, 1024], f16, tag="n1", name="n1")
            nc.vector.tensor_tensor_scan(
                n1[:], dm_t[:], m1[:],
                initial=0.0, op0=MULT, op1=ADD,
            )
            live[(k, "n1")] = n1

        def fmm1(k):
            # mm-chain final stage (replaces scan1+fmms for late groups,
            # where Act is idle): q = contraction over w first (output
            # un-transposed [h, (t,w)]), evac to fp16, then the h-direction
            # discounted sum is a matmul with the same L^T constant.
            m1 = live.pop((k, "m1"))
            q = pspool.tile([128, 1024], f32, tag="st", name="q")
            for tl in range(TG):
                nc.tensor.matmul(
                    q[:, tl * 128 : (tl + 1) * 128],
                    lhsT=m1[:, tl * 128 : (tl + 1) * 128],
                    rhs=fr_t[:],
                    skip_group_check=True,
                )
            qe = n1pool.tile([128, 1024], f16, tag="n1", name="qe")
            nc.scalar.copy(qe[:], q[:])
            live[(k, "qe")] = qe

        def fmm2(k):
            qe = live.pop((k, "qe"))
            pf = pspool.tile([128, 1024], f32, tag="st", name="pf")
            for tl in range(TG):
                nc.tensor.matmul(
                    pf[:, tl * 128 : (tl + 1) * 128],
                    lhsT=fr_t[:],
                    rhs=qe[:, tl * 128 : (tl + 1) * 128],
                    skip_group_check=True,
                )
            live[(k, "pf")] = pf

        def fmms(k):
            n1 = live.pop((k, "n1"))
            pf = pspool.tile([128, 1024], f32, tag="st", name="pf")
            for tl in range(TG):
                nc.tensor.matmul(
                    pf[:, tl * 128 : (tl + 1) * 128],
                    lhsT=n1[:, tl * 128 : (tl + 1) * 128],
                    rhs=fr_t[:],
                    skip_group_check=True,
                )
            live[(k, "pf")] = pf

        def evac_out(k):
            b, g = divmod(k, NG)
            t0 = g * TG
            pf = live.pop((k, "pf"))
            stg = stgpool.tile([128, 1024], f16, tag="stg", name="stg")
            nc.scalar.copy(stg[:], pf[:])
            dst = outp[b, :, t0 : t0 + TG, :]
            nc.sync.dma_start(
                dst, stg[:].rearrange("p (t w) -> p t w", t=TG)
            )

        # ---- schedule ----
        # head: constants + x(b0)+x(b1) + einsum(b0)
        # first 32h chunk split into 8h quarters so the einsum starts
        # as soon as the first 8 rows land
        xh = {}
        for q in range(4):
            xt_ = xpool.tile([C, 8 * W], f16, tag="x", name=f"xq{q}")
            src = xs[0, :, q * 8 : q * 8 + 8, :]
            nc.sync.dma_start(xt_[:], src.rearrange("c h w -> c (h w)"))
            xh[q] = xt_
        for hb in range(1, NHB):
            load_x(0, hb)
        load_stage_consts()
        for hb in range(NHB):
            load_x(1, hb)
        make_bp(0)
        for hb in range(NHB):
            for u in range(NU):
                n = hb * NU + u
                einsum_unit(0, hb, u, "v" if n % 2 == 0 else "s",
                            tag="st" if n % 2 else "pe", ksplit=True,
                            xt=xh[u] if hb == 0 else None,
                            ubase=u if hb == 0 else 0)
        s3mms(0)

        # steady: 16 stage ticks + 3 drain ticks; einsum(b+1) and x
        # prefetch ride inside the ticks.  Emission order within a tick is
        # readiness order (oldest group first) so in-order engines don't
        # stall on newest dependencies.
        for k in range(NT + 3):
            b, g = divmod(min(k, NT - 1), NG)
            nb = b + 1 if k < NT else BPC
            if k < NT and g == 0 and nb < BPC:
                make_bp(nb)
            _UA = [(0, 2), (4, 6), (8, 10), (12, 16)]
            _UB = [(2, 4), (6, 8), (10, 12), (16, 16)]
            if nb < BPC:
                for uu in range(*_UA[g]):
                    einsum_unit(nb, uu // NU, uu % NU,
                                "v" if uu >= 14 else "s")
            if 0 <= k - 3 < NT:
                if k - 3 >= KCONV:
                    fmm1(k - 3)
                else:
                    scan1(k - 3)
            if 0 <= k - 2 < NT:
                s2mms(k - 2)
            if 0 <= k - 1 < NT:
                if k - 1 >= KC1:
                    s3mm2_mul2(k - 1)
                else:
                    scan3(k - 1)
                    mul2(k - 1, 0, nc.gpsimd)
                    mul2(k - 1, 1, nc.vector if k - 1 < 3 else nc.gpsimd)
            if 0 <= k - 2 < NT:
                scan2(k - 2)
                mul1(k - 2, 0, nc.vector if k - 2 < 2 else nc.gpsimd)
            if 0 < k < NT:
                if k >= KC1:
                    s3mm_chain(k)
                else:
                    s3mms(k)
            if 0 <= k - 3 < NT:
                if k - 3 >= KCONV:
                    fmm2(k - 3)
                else:
                    fmms(k - 3)
                evac_out(k - 3)
            if 0 <= k - 2 < NT:
                mul1(k - 2, 1, nc.vector)
            if nb < BPC:
                for uu in range(*_UB[g]):
                    einsum_unit(nb, uu // NU, uu % NU,
                                "v" if uu >= 14 else "s")
            if k < NT and b + 2 < BPC:
                load_x(b + 2, g)

    nc.compile()
    return nc


def _host_prep_v2(alpha_1, alpha_2, alpha_3, d):
    a1 = alpha_1.T * (C1 * d * d)
    alphaT = np.concatenate(
        [a1, alpha_2.T * C2, alpha_3.T * C3], axis=1
    ).astype(np.float16)                     # [C, 3T]

    idx = np.arange(H)
    E = idx[:, None] - idx[None, :]
    P = d ** np.maximum(E, 0)
    L = np.where(E >= 0, P, 0.0)
    Ls = np.where(E >= 1, P, 0.0)
    statT = Ls.T.astype(np.float16).copy()
    stat2T = (Ls / d).T.astype(np.float16).copy()
    frhsT = L.T.astype(np.float16).copy()
    dmask = np.full((128, 1024), d, np.float32)
    dmask[:, 0::128] = 0.0
    return alphaT, statT, stat2T, frhsT, dmask


def kernel_v2(x, alpha_1, alpha_2, alpha_3, d):
    from concourse.bass_utils import run_bass_kernel_spmd

    alphaT, statT, stat2T, frhsT, dmask = _host_prep_v2(
        alpha_1, alpha_2, alpha_3, d
    )
    x = np.ascontiguousarray(x.astype(np.float16))
    key = ("nc_v2",)
    if key not in _CACHE:
        _CACHE[key] = _build_module_v2()
    nc = _CACHE[key]

    shared = {
        "alphaT": alphaT,
        "statT": statT,
        "stat2T": stat2T,
        "frhsT": frhsT,
        "dmask": dmask,
    }
    in_maps = [
        {"xs": x[i * BPC : (i + 1) * BPC], **shared} for i in range(NCORES)
    ]
    res = run_bass_kernel_spmd(nc, in_maps, core_ids=list(range(NCORES)))
    outs = [res.results[i]["outp"] for i in range(NCORES)]
    full = np.concatenate(outs, axis=0)               # [B, H, T, W] f16
    return (
        full.transpose(0, 2, 1, 3).astype(np.float32) * np.float32(UNSCALE)
    )


# ---------------------------------------------------------------------------
# v1 fallback (non-uniform discounts): per-t stationaries, fp32 output.
# ---------------------------------------------------------------------------

def _build_module(uniform_d=False):
    import concourse.bass as bass
    import concourse.mybir as mybir
    import concourse.tile as tile
    from concourse import bacc
    from contextlib import ExitStack

    dt = mybir.dt
    f32, f16 = dt.float32, dt.float16

    nc = bacc.Bacc(
        "TRN2", target_bir_lowering=False, debug=False, num_devices=NCORES
    )
    xs = nc.declare_dram_parameter("xs", [BPC, C, H, W], f32, isOutput=False)
    alphaT = nc.declare_dram_parameter("alphaT", [128, KA * T], f16, isOutput=False)
    stat3T = nc.declare_dram_parameter("stat3T", [128, T * 128], f16, isOutput=False)
    frhs = nc.declare_dram_parameter("frhs", [128, T * 128], f16, isOutput=False)
    dmask = nc.declare_dram_parameter("dmask", [128, T * 128], f32, isOutput=False)
    out = nc.declare_dram_parameter("out", [BPC, T, H, W], f32, isOutput=True)

    HB = 32                    # h-block size for x streaming
    NHB = H // HB              # 4 h-blocks
    NPAIR = BPC // 2           # 2 b-pairs
    NG = T // 4                # 8 t-quad groups
    KCONV = 16                 # groups >= KCONV use the mm-chain final
    MULT = mybir.AluOpType.mult
    ADD = mybir.AluOpType.add
    COPY = mybir.ActivationFunctionType.Copy

    with tile.TileContext(nc) as tc, ExitStack() as ctx:
        const_pool = ctx.enter_context(tc.tile_pool(name="const", bufs=1))
        xpool = ctx.enter_context(tc.tile_pool(name="xp", bufs=4))
        bppool = ctx.enter_context(tc.tile_pool(name="bp", bufs=4))
        ypool = ctx.enter_context(tc.tile_pool(name="yp", bufs=6))
        mpool = ctx.enter_context(tc.tile_pool(name="mp", bufs=6))
        n1pool = ctx.enter_context(tc.tile_pool(name="n1", bufs=4))
        stpool = ctx.enter_context(tc.tile_pool(name="st", bufs=4))
        pspool = ctx.enter_context(tc.tile_pool(name="ps", bufs=8, space="PSUM"))

        # constants: alpha first (einsum needs it immediately); the big
        # stage constants are DMA'd after x(b0) so they don't delay it.
        alpha_t = const_pool.tile([128, KA * T], f16, tag="alpha")
        nc.sync.dma_start(alpha_t[:], alphaT[:])
        s3_t = const_pool.tile([128, T * 128], f16, tag="s3m")
        fr_t = const_pool.tile([128, T * 128], f16, tag="frm")
        dm_t = const_pool.tile([128, T * 128], f32, tag="dmm")

        def load_stage_consts_head():
            # only the first-group slices early: the bulk must not queue
            # ahead of pair-0's remaining x chunks on the FIFO DMA rings
            nc.sync.dma_start(s3_t[:, 0:512], stat3T[:, 0:512])
            nc.sync.dma_start(dm_t[:, 0:512], dmask[:, 0:512])
            nc.sync.dma_start(fr_t[:, 0:512], frhs[:, 0:512])

        def load_stage_consts_rest():
            nc.sync.dma_start(s3_t[:, 512:], stat3T[:, 512:])
            nc.sync.dma_start(dm_t[:, 512:], dmask[:, 512:])
            nc.sync.dma_start(fr_t[:, 512:], frhs[:, 512:])

        bp_tiles = {}   # pair -> [tile, tile]
        bpv = {}        # pair -> rearranged views

        def make_bp(pair):
            bp_tiles[pair] = [
                bppool.tile(
                    [128, KA * T * 128], f16, tag="bp", name=f"bp{pair}_{i}"
                )
                for i in range(2)
            ]
            bpv[pair] = [
                t[:].rearrange("p (k t h) -> p k t h", k=KA, t=T)
                for t in bp_tiles[pair]
            ]

        def einsum_units(pair):
            """Generator of closures: x-DMA + (mms, pivot-evac) units."""
            for hb in range(NHB):
                holder = {}

                def ensure_x(pair=pair, hb=hb, holder=holder):
                    if "xt" not in holder:
                        xt = xpool.tile([128, HB * W], f16, tag="x", name="xt")
                        src = xs[
                            2 * pair : 2 * pair + 2, :, hb * HB : (hb + 1) * HB, :
                        ]
                        nc.gpsimd.dma_start(
                            xt[:], src.rearrange("b c h w -> (b c) (h w)")
                        )
                        holder["xt"] = xt
                    return holder["xt"]

                joff = 0
                for ui, ng in enumerate((5, 5, 5, 5, 4, 4, 4)):
                    def unit(pair=pair, hb=hb, ng=ng, joff=joff,
                             ensure_x=ensure_x, ui=ui):
                        xt = ensure_x()
                        pts = [
                            pspool.tile(
                                [128, ng * KA * T], f32, tag="ps", name=f"pe{i}"
                            )
                            for i in range(2)
                        ]
                        for j in range(ng):
                            for par in range(2):
                                nc.tensor.matmul(
                                    pts[par][:, j * 96 : (j + 1) * 96],
                                    lhsT=xt[
                                        64 * par : 64 * par + 64,
                                        (joff + j) * W : (joff + j + 1) * W,
                                    ],
                                    rhs=alpha_t[64 * par : 64 * par + 64, :],
                                    tile_position=(64 * par, 0),
                                    skip_group_check=True,
                                )
                        h0 = hb * HB + joff
                        for par in range(2):
                            src_ap = pts[par][:].rearrange(
                                "p (j k t) -> p j k t", j=ng, k=KA
                            )
                            dst_ap = (
                                bp_tiles[pair][par][:]
                                .rearrange("p (k t h) -> p k t h", k=KA, t=T)[
                                    :, :, :, h0 : h0 + ng
                                ]
                                .rearrange("p k t j -> p j k t")
                            )
                            gi = hb * 7 + ui
                            if pair == 0 and par == 0 and gi % 3 != 2:
                                nc.vector.tensor_copy(dst_ap, src_ap)
                            else:
                                nc.scalar.copy(dst_ap, src_ap)
                    yield unit
                    joff += ng

        # per-(pair, par, group) live state for the staged pipeline
        live = {}

        def s3mms(pair, par, g):
            v = bpv[pair][par]
            t0 = 4 * g
            p3 = pspool.tile([128, 512], f32, tag="ps", name="p3")
            for tl in range(4):
                t = t0 + tl
                nc.tensor.matmul(
                    p3[:, tl * 128 : (tl + 1) * 128],
                    lhsT=s3_t[:, t * 128 : (t + 1) * 128],
                    rhs=v[:, 2, t, :],
                    skip_group_check=True,
                )
            live[(pair, par, g, "p3")] = p3

        def scan3_mul2(pair, par, g):
            v = bpv[pair][par]
            t0 = 4 * g
            p3 = live.pop((pair, par, g, "p3"))
            y3 = ypool.tile([128, 516], f16, tag="y", name="y3")
            nc.vector.tensor_tensor_scan(
                y3[:, 1:513], dm_t[:, t0 * 128 : t0 * 128 + 512], p3[:],
                initial=0.0, op0=MULT, op1=ADD,
            )
            m2 = mpool.tile([128, 512], f16, tag="m", name="m2")
            nc.gpsimd.tensor_mul(
                m2[:].rearrange("p (t h) -> p t h", t=4),
                v[:, 1, t0 : t0 + 4, :],
                y3[:, 0:512].rearrange("p (t h) -> p t h", t=4),
            )
            nc.gpsimd.memset(
                m2[:].rearrange("p (t h) -> p t h", t=4)[:, :, 0:1], 0.0
            )
            live[(pair, par, g, "m2")] = m2

        def s2mms_scan2_mul1(pair, par, g):
            v = bpv[pair][par]
            t0 = 4 * g
            m2 = live.pop((pair, par, g, "m2"))
            p2 = pspool.tile([128, 512], f32, tag="ps", name="p2")
            for tl in range(4):
                t = t0 + tl
                nc.tensor.matmul(
                    p2[:, tl * 128 : (tl + 1) * 128],
                    lhsT=s3_t[:, t * 128 : (t + 1) * 128],
                    rhs=m2[:, tl * 128 : (tl + 1) * 128],
                    skip_group_check=True,
                )
            y2 = ypool.tile([128, 516], f16, tag="y", name="y2")
            nc.vector.tensor_tensor_scan(
                y2[:, 1:513], dm_t[:, t0 * 128 : t0 * 128 + 512], p2[:],
                initial=0.0, op0=MULT, op1=ADD,
            )
            m1 = mpool.tile([128, 512], f16, tag="m", name="m1")
            eng = nc.gpsimd if g % 2 == 1 else nc.vector
            eng.tensor_mul(
                m1[:].rearrange("p (t h) -> p t h", t=4),
                v[:, 0, t0 : t0 + 4, :],
                y2[:, 0:512].rearrange("p (t h) -> p t h", t=4),
            )
            eng.memset(
                m1[:].rearrange("p (t h) -> p t h", t=4)[:, :, 0:1], 0.0
            )
            live[(pair, par, g, "m1")] = m1

        def scan1_fmms_evac(pair, par, g):
            b = 2 * pair + par
            t0 = 4 * g
            m1 = live.pop((pair, par, g, "m1"))
            n1 = n1pool.tile([128, 512], f16, tag="n1", name="n1")
            nc.vector.tensor_tensor_scan(
                n1[:], dm_t[:, t0 * 128 : t0 * 128 + 512], m1[:],
                initial=0.0, op0=MULT, op1=ADD,
            )
            pf = pspool.tile([128, 512], f32, tag="ps", name="pf")
            for tl in range(4):
                t = t0 + tl
                nc.tensor.matmul(
                    pf[:, tl * 128 : (tl + 1) * 128],
                    lhsT=n1[:, tl * 128 : (tl + 1) * 128],
                    rhs=fr_t[:, t * 128 : (t + 1) * 128],
                    skip_group_check=True,
                )
            half = g // 2
            if g % 2 == 0:
                live[(pair, par, half, "st")] = stpool.tile(
                    [128, 8 * 128], f32, tag="st", name="stg"
                )
            stg = live[(pair, par, half, "st")]
            nc.scalar.activation(
                stg[:, (g % 2) * 512 : (g % 2 + 1) * 512],
                pf[:], COPY, scale=UNSCALE,
            )
            if g % 2 == 1:
                del live[(pair, par, half, "st")]
                dst = out[b, half * 8 : half * 8 + 8, :, :]
                nc.sync.dma_start(
                    dst.rearrange("t h w -> h t w"),
                    stg[:].rearrange("p (t w) -> p t w", t=8),
                )

        def stage_ticks(pair, extra=None):
            for k in range(NG + 3):
                for par in range(2):
                    if k < NG:
                        s3mms(pair, par, k)
                    if 0 <= k - 1 < NG:
                        scan3_mul2(pair, par, k - 1)
                    if 0 <= k - 2 < NG:
                        s2mms_scan2_mul1(pair, par, k - 2)
                    if 0 <= k - 3 < NG:
                        scan1_fmms_evac(pair, par, k - 3)
                if extra is not None:
                    for _ in range(3):
                        u = next(extra, None)
                        if u is not None:
                            u()

        make_bp(0)
        units0 = list(einsum_units(0))
        units0[0]()
        load_stage_consts_head()
        for u in units0[1:]:
            u()
        load_stage_consts_rest()
        make_bp(1)
        it1 = iter(list(einsum_units(1)))
        stage_ticks(0, extra=it1)
        for u in it1:
            u()
        stage_ticks(1)

    nc.compile()
    return nc


def _host_prep(alpha_1, alpha_2, alpha_3, discount):
    ds = np.asarray(discount, dtype=np.float64).reshape(T)
    # stage-2's (d*Ls) and the final-matmul's (d*L^T) d-factors are folded
    # into the alpha_1 columns (everything downstream of them is linear).
    a1scaled = alpha_1.T * (C1 * ds[None, :] ** 2)
    alphaT = np.concatenate(
        [a1scaled, alpha_2.T * C2, alpha_3.T * C3], axis=1
    ).astype(np.float16)
    alphaT_dup = np.concatenate([alphaT, alphaT], axis=0)  # [128, 96]

    idx = np.arange(H)
    E = idx[:, None] - idx[None, :]
    stat3T = np.zeros((128, T * 128), np.float16)
    frhs = np.zeros((128, T * 128), np.float16)
    dmask = np.zeros((128, T * 128), np.float32)
    for t in range(T):
        d = ds[t]
        P = d ** np.maximum(E, 0)
        L = np.where(E >= 0, P, 0.0)
        Ls = np.where(E >= 1, P, 0.0)
        sl = slice(t * 128, (t + 1) * 128)
        stat3T[:, sl] = Ls.T.astype(np.float16)
        frhs[:, sl] = L.T.astype(np.float16)
        dmask[:, sl] = np.float32(d)
        dmask[:, t * 128] = 0.0
    return alphaT_dup, stat3T, frhs, dmask


def _kernel_v1(x, alpha_1, alpha_2, alpha_3, discount):
    from concourse.bass_utils import run_bass_kernel_spmd

    alphaT_dup, stat3T, frhs, dmask = _host_prep(
        alpha_1, alpha_2, alpha_3, discount
    )
    key = ("nc", False)
    if key not in _CACHE:
        _CACHE[key] = _build_module(uniform_d=False)
    nc = _CACHE[key]

    shared = {
        "alphaT": alphaT_dup,
        "stat3T": stat3T,
        "frhs": frhs,
        "dmask": dmask,
    }
    in_maps = [
        {"xs": x[i * BPC : (i + 1) * BPC], **shared} for i in range(NCORES)
    ]
    res = run_bass_kernel_spmd(nc, in_maps, core_ids=list(range(NCORES)))
    outs = [res.results[i]["out"] for i in range(NCORES)]
    return np.concatenate(outs, axis=0).astype(np.float32)


def kernel(x, alpha_1, alpha_2, alpha_3, discount):
    x = np.ascontiguousarray(np.asarray(x, dtype=np.float32))
    a1 = np.asarray(alpha_1, np.float32)
    a2 = np.asarray(alpha_2, np.float32)
    a3 = np.asarray(alpha_3, np.float32)
    ds = np.asarray(discount, dtype=np.float64).reshape(T)

    if np.all(ds == ds[0]):
        return kernel_v2(x, a1, a2, a3, float(ds[0]))
    return _kernel_v1(x, a1, a2, a3, discount)


if __name__ == "__main__":
    import reference as ref

    inputs = {k: np.asarray(v) for k, v in ref.setup_inputs().items()}
    got = kernel(**inputs)
    print("kernel output shape:", got.shape, got.dtype)


# revision 80
# speedup vs baseline: 1.0218x; 1.0047x over previous
"""Trainium2 Bass kernel for nn_FISLayerParameterSharingV2.

Math: dcumsum along an axis with discount d is multiplication by a lower
triangular matrix L[i,j] = d^(i-j).  With H = W = 128 the whole per-(b,t)
chain is expressible as 128x128 matmuls + elementwise products:

    s3  = Ls Z3 Ls^T          (Ls strict lower triangular)
    s2  = Ls (Z2*s3) Ls^T
    out = L  (Z1*s2) L^T      (L inclusive lower triangular)

v2 layout strategy (per core; B sharded 4 per core over 8 cores), for the
uniform-discount case (discount[t] identical for all t, which the reference
setup always produces):

  *  x streamed per (b, 32h-block) as [64c, 32*128] fp16 tiles; einsum
     matmul(lhsT=x[c, w-slice], rhs=alphaT[c, 3T]) emits Z^T tiles
     [w, 3T] per (b,h) which are pivoted into a per-b SBUF buffer
     Bp[w, (k,t,h)] fp16 by Act/DVE copies.
  *  All t share one stationary Ls^T / L^T matrix (uniform d), so the
     big per-t constant tables shrink to single 128x128 tiles and the
     scan multiplier mask to one [128, 1024] tile.
  *  Stages run in transposed space [w, h] on 8-t groups: left Ls
     multiply = 8 PE matmuls into a 2-bank PSUM tile [128, 1024]; right
     Ls^T multiply = one discounted inclusive scan over the 1024-wide
     free dim on DVE (resets at each 128 boundary via the mask), which
     also evacuates PSUM->SBUF fp16.  Elementwise Z*s products run on
     GpSimd/DVE (SBUF only).  Strict shift = AP offset + column memset.
  *  Final stage matmul (lhsT=n1, rhs=L^T) un-transposes to [h, (t,w)];
     Act evacuates PSUM to a fp16 SBUF staging tile which DMAs to a
     DRAM output laid out [b, h, t, w] with 2KB contiguous runs; the
     host transposes to [b, t, h, w] and applies the 2^20 unscale.
  *  fp16 storage everywhere with power-of-2 prescales folded into the
     alphas (the scans amplify ~50x per stage and would overflow fp16).

A non-uniform discount falls back to the per-t v1 module (kept below).
"""

import sys
import numpy as np

for _p in ("/opt/trn_rl_repo",):
    if _p not in sys.path:
        sys.path.insert(0, _p)

B, T, C, H, W = 32, 32, 64, 128, 128
NCORES = 8
BPC = B // NCORES          # batches per core
KA = 3                     # number of alphas
C1, C2, C3 = 2.0 ** -8, 2.0 ** -6, 2.0 ** -6
UNSCALE = 1.0 / (C1 * C2 * C3)

_CACHE = {}


def _build_module_v2():
    """Uniform-discount module: shared stationaries, 8t-merged scans,
    fp16 [b,h,t,w] DRAM output."""
    import concourse.bass as bass
    import concourse.mybir as mybir
    import concourse.tile as tile
    from concourse import bacc
    from contextlib import ExitStack

    dt = mybir.dt
    f32, f16 = dt.float32, dt.float16

    nc = bacc.Bacc(
        "TRN2", target_bir_lowering=False, debug=False, num_devices=NCORES
    )
    xs = nc.declare_dram_parameter("xs", [BPC, C, H, W], f16, isOutput=False)
    alphaT = nc.declare_dram_parameter("alphaT", [C, KA * T], f16, isOutput=False)
    statT = nc.declare_dram_parameter("statT", [128, 128], f16, isOutput=False)
    stat2T = nc.declare_dram_parameter("stat2T", [128, 128], f16, isOutput=False)
    frhsT = nc.declare_dram_parameter("frhsT", [128, 128], f16, isOutput=False)
    dmask = nc.declare_dram_parameter("dmask", [128, 1024], f32, isOutput=False)
    outp = nc.declare_dram_parameter("outp", [BPC, H, T, W], f16, isOutput=True)

    HB = 32                    # h-block size for x streaming
    NHB = H // HB              # 4 h-blocks per b
    TG = 8                     # t-group size per stage tick
    NG = T // TG               # 4 t-groups per b
    NT = BPC * NG              # 16 stage ticks
    JU = 8                     # einsum h-slices per psum unit
    NU = HB // JU              # 2 einsum units per (b, hb)
    KCONV = 16                # groups >= KCONV use the mm-chain final
    MULT = mybir.AluOpType.mult
    ADD = mybir.AluOpType.add

    with tile.TileContext(nc) as tc, ExitStack() as ctx:
        const_pool = ctx.enter_context(tc.tile_pool(name="const", bufs=1))
        xpool = ctx.enter_context(tc.tile_pool(name="xp", bufs=8))
        bppool = ctx.enter_context(tc.tile_pool(name="bp", bufs=3))
        ypool = ctx.enter_context(tc.tile_pool(name="yp", bufs=4))
        mpool = ctx.enter_context(tc.tile_pool(name="mp", bufs=4))
        n1pool = ctx.enter_context(tc.tile_pool(name="n1", bufs=3))
        stgpool = ctx.enter_context(tc.tile_pool(name="stg", bufs=3))
        pspool = ctx.enter_context(tc.tile_pool(name="ps", bufs=2, space="PSUM"))

        atlw = const_pool.tile([1, 8], f32, tag="atlw")
        nc.scalar.memzero(atlw[:])
        alpha_t = const_pool.tile([C, KA * T], f16, tag="alpha")
        nc.gpsimd.dma_start(alpha_t[:], alphaT[:])
        stat_t = const_pool.tile([128, 128], f16, tag="stat")
        stat2_t = const_pool.tile([128, 128], f16, tag="stat2")
        fr_t = const_pool.tile([128, 128], f16, tag="fr")
        dm_t = const_pool.tile([128, 1024], f32, tag="dm")

        def load_stage_consts():
            nc.gpsimd.dma_start(stat_t[:], statT[:])
            nc.gpsimd.dma_start(stat2_t[:], stat2T[:])
            nc.gpsimd.dma_start(fr_t[:], frhsT[:])
            nc.gpsimd.dma_start(dm_t[:], dmask[:])

        x_tiles = {}

        def load_x(b, hb):
            xt = xpool.tile([C, HB * W], f16, tag="x", name=f"xt{b}_{hb}")
            src = xs[b, :, hb * HB : (hb + 1) * HB, :]
            nc.sync.dma_start(xt[:], src.rearrange("c h w -> c (h w)"))
            x_tiles[(b, hb)] = xt

        bp_tiles = {}
        bpv = {}

        def make_bp(b):
            t_ = bppool.tile(
                [128, KA * T * 128], f16, tag="bp", name=f"bp{b}"
            )
            bp_tiles[b] = t_
            bpv[b] = t_[:].rearrange("p (k t h) -> p k t h", k=KA, t=T)

        def einsum_unit(b, hb, u, evac_eng, tag="pe", ksplit=False,
                        xt=None, ubase=0):
            """One einsum unit: 8 matmuls [w,96] + one pivot evacuation."""
            if xt is None:
                xt = x_tiles[(b, hb)]
            pts = pspool.tile([128, JU * KA * T], f32, tag=tag, name="pe")
            for j in range(JU):
                jj = (u - ubase) * JU + j
                nc.tensor.matmul(
                    pts[:, j * 96 : (j + 1) * 96],
                    lhsT=xt[:, jj * W : (jj + 1) * W],
                    rhs=alpha_t[:],
                    skip_group_check=True,
                )
            h0 = hb * HB + u * JU
            src_ap = pts[:].rearrange("p (j k t) -> p j k t", j=JU, k=KA)
            dst_ap = (
                bpv[b][:, :, :, h0 : h0 + JU]
                .rearrange("p k t j -> p j k t")
            )
            if ksplit:
                # z3 pieces stream on DVE (fast, unblocks stage-1 early);
                # z1/z2 pieces stream on Act in parallel.
                nc.vector.tensor_copy(dst_ap[:, :, 2:3], src_ap[:, :, 2:3])
                nc.scalar.copy(dst_ap[:, :, 0:2], src_ap[:, :, 0:2])
            elif evac_eng == "v":
                nc.vector.tensor_copy(dst_ap, src_ap)
            else:
                nc.scalar.copy(dst_ap, src_ap)
            return None

        live = {}

        def s3mms(k):
            b, g = divmod(k, NG)
            v = bpv[b]
            p3 = pspool.tile([128, 1024], f32, tag="st", name="p3")
            for tl in range(TG):
                t_ = g * TG + tl
                nc.tensor.matmul(
                    p3[:, tl * 128 : (tl + 1) * 128],
                    lhsT=stat_t[:],
                    rhs=v[:, 2, t_, :],
                    skip_group_check=True,
                )
            live[(k, "p3")] = p3

        KC1 = 99                   # disabled: trades DVE scan for DVE psum-mul (no net)

        def s3mm_chain(k):
            # w-contraction first, output un-transposed [h, (t,w)] in PSUM,
            # evacuated fp16 by Act (idle in the tail region).
            b, g = divmod(k, NG)
            v = bpv[b]
            q = pspool.tile([128, 1024], f32, tag="st", name="q3")
            for tl in range(TG):
                t_ = g * TG + tl
                nc.tensor.matmul(
                    q[:, tl * 128 : (tl + 1) * 128],
                    lhsT=v[:, 2, t_, :],
                    rhs=stat_t[:],
                    skip_group_check=True,
                )
            e3 = n1pool.tile([128, 1024], f16, tag="n1", name="e3")
            nc.scalar.copy(e3[:], q[:])
            live[(k, "e3")] = e3

        def s3mm2_mul2(k):
            # h-direction strict discounted sum as a matmul with (Ls/d)^T
            # (the /d matches the scan path's shifted-read semantics), then
            # m2 = z2 * s3 with the DVE mul reading PSUM directly.  The
            # strict matrix zeroes h=0, so no column memset is needed.
            b, g = divmod(k, NG)
            v = bpv[b][:, :, g * TG : g * TG + TG, :]
            e3 = live.pop((k, "e3"))
            s3p = pspool.tile([128, 1024], f32, tag="st", name="s3p")
            for tl in range(TG):
                nc.tensor.matmul(
                    s3p[:, tl * 128 : (tl + 1) * 128],
                    lhsT=e3[:, tl * 128 : (tl + 1) * 128],
                    rhs=stat2_t[:],
                    skip_group_check=True,
                )
            m2 = mpool.tile([128, 1024], f16, tag="m", name="m2")
            nc.vector.tensor_mul(
                m2[:].rearrange("p (t h) -> p t h", t=TG),
                v[:, 1, :, :],
                s3p[:].rearrange("p (t h) -> p t h", t=TG),
            )
            live[(k, "m2")] = m2

        def scan3(k):
            p3 = live.pop((k, "p3"))
            y3 = ypool.tile([128, 1025], f16, tag="y", name="y3")
            nc.vector.tensor_tensor_scan(
                y3[:, 1:1025], dm_t[:], p3[:],
                initial=0.0, op0=MULT, op1=ADD,
            )
            live[(k, "y3")] = y3

        def _mul_half(v, ki, m, y, half, eng):
            # m[half] = z_k ⊙ shifted-scan, one 512-wide half (4 t-blocks)
            t04 = slice(half * 4, half * 4 + 4)
            mv = m[:].rearrange("p (t h) -> p t h", t=TG)[:, t04, :]
            eng.tensor_mul(
                mv,
                v[:, ki, t04, :],
                y[:, half * 512 : half * 512 + 512].rearrange(
                    "p (t h) -> p t h", t=4
                ),
            )
            eng.memset(mv[:, :, 0:1], 0.0)

        def mul2(k, half, eng):
            b, g = divmod(k, NG)
            v = bpv[b][:, :, g * TG : g * TG + TG, :]
            if (k, "m2") not in live:
                live[(k, "m2")] = mpool.tile(
                    [128, 1024], f16, tag="m", name="m2"
                )
            _mul_half(v, 1, live[(k, "m2")], live[(k, "y3")], half, eng)
            if half == 1:
                del live[(k, "y3")]

        def s2mms(k):
            m2 = live.pop((k, "m2"))
            p2 = pspool.tile([128, 1024], f32, tag="st", name="p2")
            for tl in range(TG):
                nc.tensor.matmul(
                    p2[:, tl * 128 : (tl + 1) * 128],
                    lhsT=stat_t[:],
                    rhs=m2[:, tl * 128 : (tl + 1) * 128],
                    skip_group_check=True,
                )
            live[(k, "p2")] = p2

        def scan2(k):
            p2 = live.pop((k, "p2"))
            y2 = ypool.tile([128, 1025], f16, tag="y", name="y2")
            nc.vector.tensor_tensor_scan(
                y2[:, 1:1025], dm_t[:], p2[:],
                initial=0.0, op0=MULT, op1=ADD,
            )
            live[(k, "y2")] = y2

        def mul1(k, half, eng):
            b, g = divmod(k, NG)
            v = bpv[b][:, :, g * TG : g * TG + TG, :]
            if (k, "m1") not in live:
                live[(k, "m1")] = mpool.tile(
                    [128, 1024], f16, tag="m", name="m1"
                )
            _mul_half(v, 0, live[(k, "m1")], live[(k, "y2")], half, eng)
            if half == 1:
                del live[(k, "y2")]

        def scan1(k):
            m1 = live.pop((k, "m1"))
            n1 = n1pool.tile([128, 1024], f16, tag="n1", name="n1")
            nc.vector.tensor_tensor_scan(
                n1[:], dm_t[:], m1[:],
                initial=0.0, op0=MULT, op1=ADD,
            )
            live[(k, "n1")] = n1

        def fmm1(k):
            # mm-chain final stage (replaces scan1+fmms for late groups,
            # where Act is idle): q = contraction over w first (output
            # un-transposed [h, (t,w)]), evac to fp16, then the h-direction
            # discounted sum is a matmul with the same L^T constant.
            m1 = live.pop((k, "m1"))
            q = pspool.tile([128, 1024], f32, tag="st", name="q")
            for tl in range(TG):
                nc.tensor.matmul(
                    q[:, tl * 128 : (tl + 1) * 128],
                    lhsT=m1[:, tl * 128 : (tl + 1) * 128],
                    rhs=fr_t[:],
                    skip_group_check=True,
                )
            qe = n1pool.tile([128, 1024], f16, tag="n1", name="qe")
            nc.scalar.copy(qe[:], q[:])
            live[(k, "qe")] = qe

        def fmm2(k):
            qe = live.pop((k, "qe"))
            pf = pspool.tile([128, 1024], f32, tag="st", name="pf")
            for tl in range(TG):
                nc.tensor.matmul(
                    pf[:, tl * 128 : (tl + 1) * 128],
                    lhsT=fr_t[:],
                    rhs=qe[:, tl * 128 : (tl + 1) * 128],
                    skip_group_check=True,
                )
            live[(k, "pf")] = pf

        def fmms(k):
            n1 = live.pop((k, "n1"))
            pf = pspool.tile([128, 1024], f32, tag="st", name="pf")
            for tl in range(TG):
                nc.tensor.matmul(
                    pf[:, tl * 128 : (tl + 1) * 128],
                    lhsT=n1[:, tl * 128 : (tl + 1) * 128],
                    rhs=fr_t[:],
                    skip_group_check=True,
                )
            live[(k, "pf")] = pf

        def evac_out(k):
            b, g = divmod(k, NG)
            t0 = g * TG
            pf = live.pop((k, "pf"))
            stg = stgpool.tile([128, 1024], f16, tag="stg", name="stg")
            nc.scalar.copy(stg[:], pf[:])
            dst = outp[b, :, t0 : t0 + TG, :]
            nc.sync.dma_start(
                dst, stg[:].rearrange("p (t w) -> p t w", t=TG)
            )

        # ---- schedule ----
        # head: constants + x(b0)+x(b1) + einsum(b0)
        # first 32h chunk split into 8h quarters so the einsum starts
        # as soon as the first 8 rows land
        xh = {}
        for q in range(4):
            xt_ = xpool.tile([C, 8 * W], f16, tag="x", name=f"xq{q}")
            src = xs[0, :, q * 8 : q * 8 + 8, :]
            nc.sync.dma_start(xt_[:], src.rearrange("c h w -> c (h w)"))
            xh[q] = xt_
        for hb in (1, 2):
            for half in range(2):
                xt_ = xpool.tile([C, 16 * W], f16, tag="x",
                                 name=f"xh{hb}_{half}")
                src = xs[0, :, hb * HB + half * 16 : hb * HB + half * 16 + 16, :]
                nc.sync.dma_start(xt_[:], src.rearrange("c h w -> c (h w)"))
                xh[(hb, half)] = xt_
        for hb in range(3, NHB):
            load_x(0, hb)
        load_stage_consts()
        for hb in range(NHB):
            load_x(1, hb)
        make_bp(0)
        for hb in range(NHB):
            for u in range(NU):
                n = hb * NU + u
                einsum_unit(0, hb, u, "v" if n % 2 == 0 else "s",
                            tag="st" if n % 2 else "pe", ksplit=True,
                            xt=(xh[u] if hb == 0 else
                                xh[(hb, u // 2)] if hb in (1, 2) else None),
                            ubase=(u if hb == 0 else
                                   (u // 2) * 2 if hb in (1, 2) else 0))
        s3mms(0)

        # steady: 16 stage ticks + 3 drain ticks; einsum(b+1) and x
        # prefetch ride inside the ticks.  Emission order within a tick is
        # readiness order (oldest group first) so in-order engines don't
        # stall on newest dependencies.
        for k in range(NT + 3):
            b, g = divmod(min(k, NT - 1), NG)
            nb = b + 1 if k < NT else BPC
            if k < NT and g == 0 and nb < BPC:
                make_bp(nb)
            _UA = [(0, 2), (4, 6), (8, 10), (12, 16)]
            _UB = [(2, 4), (6, 8), (10, 12), (16, 16)]
            if nb < BPC:
                for uu in range(*_UA[g]):
                    einsum_unit(nb, uu // NU, uu % NU,
                                "v" if uu >= 14 else "s")
            if 0 <= k - 3 < NT:
                if k - 3 >= KCONV:
                    fmm1(k - 3)
                else:
                    scan1(k - 3)
            if 0 <= k - 2 < NT:
                s2mms(k - 2)
            if 0 <= k - 1 < NT:
                if k - 1 >= KC1:
                    s3mm2_mul2(k - 1)
                else:
                    scan3(k - 1)
                    mul2(k - 1, 0, nc.gpsimd)
                    mul2(k - 1, 1, nc.vector if k - 1 < 3 else nc.gpsimd)
            if 0 <= k - 2 < NT:
                scan2(k - 2)
                mul1(k - 2, 0, nc.vector if k - 2 < 2 else nc.gpsimd)
            if 0 < k < NT:
                if k >= KC1:
                    s3mm_chain(k)
                else:
                    s3mms(k)
            if 0 <= k - 3 < NT:
                if k - 3 >= KCONV:
                    fmm2(k - 3)
                else:
                    fmms(k - 3)
                evac_out(k - 3)
            if 0 <= k - 2 < NT:
                mul1(k - 2, 1, nc.vector)
            if nb < BPC:
                for uu in range(*_UB[g]):
                    einsum_unit(nb, uu // NU, uu % NU,
                                "v" if uu >= 14 else "s")
            if k < NT and b + 2 < BPC:
                load_x(b + 2, g)

    nc.compile()
    return nc


def _host_prep_v2(alpha_1, alpha_2, alpha_3, d):
    a1 = alpha_1.T * (C1 * d * d)
    alphaT = np.concatenate(
        [a1, alpha_2.T * C2, alpha_3.T * C3], axis=1
    ).astype(np.float16)                     # [C, 3T]

    idx = np.arange(H)
    E = idx[:, None] - idx[None, :]
    P = d ** np.maximum(E, 0)
    L = np.where(E >= 0, P, 0.0)
    Ls = np.where(E >= 1, P, 0.0)
    statT = Ls.T.astype(np.float16).copy()
    stat2T = (Ls / d).T.astype(np.float16).copy()
    frhsT = L.T.astype(np.float16).copy()
    dmask = np.full((128, 1024), d, np.float32)
    dmask[:, 0::128] = 0.0
    return alphaT, statT, stat2T, frhsT, dmask


def kernel_v2(x, alpha_1, alpha_2, alpha_3, d):
    from concourse.bass_utils import run_bass_kernel_spmd

    alphaT, statT, stat2T, frhsT, dmask = _host_prep_v2(
        alpha_1, alpha_2, alpha_3, d
    )
    x = np.ascontiguousarray(x.astype(np.float16))
    key = ("nc_v2",)
    if key not in _CACHE:
        _CACHE[key] = _build_module_v2()
    nc = _CACHE[key]

    shared = {
        "alphaT": alphaT,
        "statT": statT,
        "stat2T": stat2T,
        "frhsT": frhsT,
        "dmask": dmask,
    }
    in_maps = [
        {"xs": x[i * BPC : (i + 1) * BPC], **shared} for i in range(NCORES)
    ]
    res = run_bass_kernel_spmd(nc, in_maps, core_ids=list(range(NCORES)))
    outs = [res.results[i]["outp"] for i in range(NCORES)]
    full = np.concatenate(outs, axis=0)               # [B, H, T, W] f16
    return (
        full.transpose(0, 2, 1, 3).astype(np.float32) * np.float32(UNSCALE)
    )


# ---------------------------------------------------------------------------
# v1 fallback (non-uniform discounts): per-t stationaries, fp32 output.
# ---------------------------------------------------------------------------

def _build_module(uniform_d=False):
    import concourse.bass as bass
    import concourse.mybir as mybir
    import concourse.tile as tile
    from concourse import bacc
    from contextlib import ExitStack

    dt = mybir.dt
    f32, f16 = dt.float32, dt.float16

    nc = bacc.Bacc(
        "TRN2", target_bir_lowering=False, debug=False, num_devices=NCORES
    )
    xs = nc.declare_dram_parameter("xs", [BPC, C, H, W], f32, isOutput=False)
    alphaT = nc.declare_dram_parameter("alphaT", [128, KA * T], f16, isOutput=False)
    stat3T = nc.declare_dram_parameter("stat3T", [128, T * 128], f16, isOutput=False)
    frhs = nc.declare_dram_parameter("frhs", [128, T * 128], f16, isOutput=False)
    dmask = nc.declare_dram_parameter("dmask", [128, T * 128], f32, isOutput=False)
    out = nc.declare_dram_parameter("out", [BPC, T, H, W], f32, isOutput=True)

    HB = 32                    # h-block size for x streaming
    NHB = H // HB              # 4 h-blocks
    NPAIR = BPC // 2           # 2 b-pairs
    NG = T // 4                # 8 t-quad groups
    KCONV = 16                # groups >= KCONV use the mm-chain final
    MULT = mybir.AluOpType.mult
    ADD = mybir.AluOpType.add
    COPY = mybir.ActivationFunctionType.Copy

    with tile.TileContext(nc) as tc, ExitStack() as ctx:
        const_pool = ctx.enter_context(tc.tile_pool(name="const", bufs=1))
        xpool = ctx.enter_context(tc.tile_pool(name="xp", bufs=4))
        bppool = ctx.enter_context(tc.tile_pool(name="bp", bufs=4))
        ypool = ctx.enter_context(tc.tile_pool(name="yp", bufs=6))
        mpool = ctx.enter_context(tc.tile_pool(name="mp", bufs=6))
        n1pool = ctx.enter_context(tc.tile_pool(name="n1", bufs=4))
        stpool = ctx.enter_context(tc.tile_pool(name="st", bufs=4))
        pspool = ctx.enter_context(tc.tile_pool(name="ps", bufs=8, space="PSUM"))

        # constants: alpha first (einsum needs it immediately); the big
        # stage constants are DMA'd after x(b0) so they don't delay it.
        alpha_t = const_pool.tile([128, KA * T], f16, tag="alpha")
        nc.sync.dma_start(alpha_t[:], alphaT[:])
        s3_t = const_pool.tile([128, T * 128], f16, tag="s3m")
        fr_t = const_pool.tile([128, T * 128], f16, tag="frm")
        dm_t = const_pool.tile([128, T * 128], f32, tag="dmm")

        def load_stage_consts_head():
            # only the first-group slices early: the bulk must not queue
            # ahead of pair-0's remaining x chunks on the FIFO DMA rings
            nc.sync.dma_start(s3_t[:, 0:512], stat3T[:, 0:512])
            nc.sync.dma_start(dm_t[:, 0:512], dmask[:, 0:512])
            nc.sync.dma_start(fr_t[:, 0:512], frhs[:, 0:512])

        def load_stage_consts_rest():
            nc.sync.dma_start(s3_t[:, 512:], stat3T[:, 512:])
            nc.sync.dma_start(dm_t[:, 512:], dmask[:, 512:])
            nc.sync.dma_start(fr_t[:, 512:], frhs[:, 512:])

        bp_tiles = {}   # pair -> [tile, tile]
        bpv = {}        # pair -> rearranged views

        def make_bp(pair):
            bp_tiles[pair] = [
                bppool.tile(
                    [128, KA * T * 128], f16, tag="bp", name=f"bp{pair}_{i}"
                )
                for i in range(2)
            ]
            bpv[pair] = [
                t[:].rearrange("p (k t h) -> p k t h", k=KA, t=T)
                for t in bp_tiles[pair]
            ]

        def einsum_units(pair):
            """Generator of closures: x-DMA + (mms, pivot-evac) units."""
            for hb in range(NHB):
                holder = {}

                def ensure_x(pair=pair, hb=hb, holder=holder):
                    if "xt" not in holder:
                        xt = xpool.tile([128, HB * W], f16, tag="x", name="xt")
                        src = xs[
                            2 * pair : 2 * pair + 2, :, hb * HB : (hb + 1) * HB, :
                        ]
                        nc.gpsimd.dma_start(
                            xt[:], src.rearrange("b c h w -> (b c) (h w)")
                        )
                        holder["xt"] = xt
                    return holder["xt"]

                joff = 0
                for ui, ng in enumerate((5, 5, 5, 5, 4, 4, 4)):
                    def unit(pair=pair, hb=hb, ng=ng, joff=joff,
                             ensure_x=ensure_x, ui=ui):
                        xt = ensure_x()
                        pts = [
                            pspool.tile(
                                [128, ng * KA * T], f32, tag="ps", name=f"pe{i}"
                            )
                            for i in range(2)
                        ]
                        for j in range(ng):
                            for par in range(2):
                                nc.tensor.matmul(
                                    pts[par][:, j * 96 : (j + 1) * 96],
                                    lhsT=xt[
                                        64 * par : 64 * par + 64,
                                        (joff + j) * W : (joff + j + 1) * W,
                                    ],
                                    rhs=alpha_t[64 * par : 64 * par + 64, :],
                                    tile_position=(64 * par, 0),
                                    skip_group_check=True,
                                )
                        h0 = hb * HB + joff
                        for par in range(2):
                            src_ap = pts[par][:].rearrange(
                                "p (j k t) -> p j k t", j=ng, k=KA
                            )
                            dst_ap = (
                                bp_tiles[pair][par][:]
                                .rearrange("p (k t h) -> p k t h", k=KA, t=T)[
                                    :, :, :, h0 : h0 + ng
                                ]
                                .rearrange("p k t j -> p j k t")
                            )
                            gi = hb * 7 + ui
                            if pair == 0 and par == 0 and gi % 3 != 2:
                                nc.vector.tensor_copy(dst_ap, src_ap)
                            else:
                                nc.scalar.copy(dst_ap, src_ap)
                    yield unit
                    joff += ng

        # per-(pair, par, group) live state for the staged pipeline
        live = {}

        def s3mms(pair, par, g):
            v = bpv[pair][par]
            t0 = 4 * g
            p3 = pspool.tile([128, 512], f32, tag="ps", name="p3")
            for tl in range(4):
                t = t0 + tl
                nc.tensor.matmul(
                    p3[:, tl * 128 : (tl + 1) * 128],
                    lhsT=s3_t[:, t * 128 : (t + 1) * 128],
                    rhs=v[:, 2, t, :],
                    skip_group_check=True,
                )
            live[(pair, par, g, "p3")] = p3

        def scan3_mul2(pair, par, g):
            v = bpv[pair][par]
            t0 = 4 * g
            p3 = live.pop((pair, par, g, "p3"))
            y3 = ypool.tile([128, 516], f16, tag="y", name="y3")
            nc.vector.tensor_tensor_scan(
                y3[:, 1:513], dm_t[:, t0 * 128 : t0 * 128 + 512], p3[:],
                initial=0.0, op0=MULT, op1=ADD,
            )
            m2 = mpool.tile([128, 512], f16, tag="m", name="m2")
            nc.gpsimd.tensor_mul(
                m2[:].rearrange("p (t h) -> p t h", t=4),
                v[:, 1, t0 : t0 + 4, :],
                y3[:, 0:512].rearrange("p (t h) -> p t h", t=4),
            )
            nc.gpsimd.memset(
                m2[:].rearrange("p (t h) -> p t h", t=4)[:, :, 0:1], 0.0
            )
            live[(pair, par, g, "m2")] = m2

        def s2mms_scan2_mul1(pair, par, g):
            v = bpv[pair][par]
            t0 = 4 * g
            m2 = live.pop((pair, par, g, "m2"))
            p2 = pspool.tile([128, 512], f32, tag="ps", name="p2")
            for tl in range(4):
                t = t0 + tl
                nc.tensor.matmul(
                    p2[:, tl * 128 : (tl + 1) * 128],
                    lhsT=s3_t[:, t * 128 : (t + 1) * 128],
                    rhs=m2[:, tl * 128 : (tl + 1) * 128],
                    skip_group_check=True,
                )
            y2 = ypool.tile([128, 516], f16, tag="y", name="y2")
            nc.vector.tensor_tensor_scan(
                y2[:, 1:513], dm_t[:, t0 * 128 : t0 * 128 + 512], p2[:],
                initial=0.0, op0=MULT, op1=ADD,
            )
            m1 = mpool.tile([128, 512], f16, tag="m", name="m1")
            eng = nc.gpsimd if g % 2 == 1 else nc.vector
            eng.tensor_mul(
                m1[:].rearrange("p (t h) -> p t h", t=4),
                v[:, 0, t0 : t0 + 4, :],
                y2[:, 0:512].rearrange("p (t h) -> p t h", t=4),
            )
            eng.memset(
                m1[:].rearrange("p (t h) -> p t h", t=4)[:, :, 0:1], 0.0
            )
            live[(pair, par, g, "m1")] = m1

        def scan1_fmms_evac(pair, par, g):
            b = 2 * pair + par
            t0 = 4 * g
            m1 = live.pop((pair, par, g, "m1"))
            n1 = n1pool.tile([128, 512], f16, tag="n1", name="n1")
            nc.vector.tensor_tensor_scan(
                n1[:], dm_t[:, t0 * 128 : t0 * 128 + 512], m1[:],
                initial=0.0, op0=MULT, op1=ADD,
            )
            pf = pspool.tile([128, 512], f32, tag="ps", name="pf")
            for tl in range(4):
                t = t0 + tl
                nc.tensor.matmul(
                    pf[:, tl * 128 : (tl + 1) * 128],
                    lhsT=n1[:, tl * 128 : (tl + 1) * 128],
                    rhs=fr_t[:, t * 128 : (t + 1) * 128],
                    skip_group_check=True,
                )
            half = g // 2
            if g % 2 == 0:
                live[(pair, par, half, "st")] = stpool.tile(
                    [128, 8 * 128], f32, tag="st", name="stg"
                )
            stg = live[(pair, par, half, "st")]
            nc.scalar.activation(
                stg[:, (g % 2) * 512 : (g % 2 + 1) * 512],
                pf[:], COPY, scale=UNSCALE,
            )
            if g % 2 == 1:
                del live[(pair, par, half, "st")]
                dst = out[b, half * 8 : half * 8 + 8, :, :]
                nc.sync.dma_start(
                    dst.rearrange("t h w -> h t w"),
                    stg[:].rearrange("p (t w) -> p t w", t=8),
                )

        def stage_ticks(pair, extra=None):
            for k in range(NG + 3):
                for par in range(2):
                    if k < NG:
                        s3mms(pair, par, k)
                    if 0 <= k - 1 < NG:
                        scan3_mul2(pair, par, k - 1)
                    if 0 <= k - 2 < NG:
                        s2mms_scan2_mul1(pair, par, k - 2)
                    if 0 <= k - 3 < NG:
                        scan1_fmms_evac(pair, par, k - 3)
                if extra is not None:
                    for _ in range(3):
                        u = next(extra, None)
                        if u is not None:
                            u()

        make_bp(0)
        units0 = list(einsum_units(0))
        units0[0]()
        load_stage_consts_head()
        for u in units0[1:]:
            u()
        load_stage_consts_rest()
        make_bp(1)
        it1 = iter(list(einsum_units(1)))
        stage_ticks(0, extra=it1)
        for u in it1:
            u()
        stage_ticks(1)

    nc.compile()
    return nc


def _host_prep(alpha_1, alpha_2, alpha_3, discount):
    ds = np.asarray(discount, dtype=np.float64).reshape(T)
    # stage-2's (d*Ls) and the final-matmul's (d*L^T) d-factors are folded
    # into the alpha_1 columns (everything downstream of them is linear).
    a1scaled = alpha_1.T * (C1 * ds[None, :] ** 2)
    alphaT = np.concatenate(
        [a1scaled, alpha_2.T * C2, alpha_3.T * C3], axis=1
    ).astype(np.float16)
    alphaT_dup = np.concatenate([alphaT, alphaT], axis=0)  # [128, 96]

    idx = np.arange(H)
    E = idx[:, None] - idx[None, :]
    stat3T = np.zeros((128, T * 128), np.float16)
    frhs = np.zeros((128, T * 128), np.float16)
    dmask = np.zeros((128, T * 128), np.float32)
    for t in range(T):
        d = ds[t]
        P = d ** np.maximum(E, 0)
        L = np.where(E >= 0, P, 0.0)
        Ls = np.where(E >= 1, P, 0.0)
        sl = slice(t * 128, (t + 1) * 128)
        stat3T[:, sl] = Ls.T.astype(np.float16)
        frhs[:, sl] = L.T.astype(np.float16)
        dmask[:, sl] = np.float32(d)
        dmask[:, t * 128] = 0.0
    return alphaT_dup, stat3T, frhs, dmask


def _kernel_v1(x, alpha_1, alpha_2, alpha_3, discount):
    from concourse.bass_utils import run_bass_kernel_spmd

    alphaT_dup, stat3T, frhs, dmask = _host_prep(
        alpha_1, alpha_2, alpha_3, discount
    )
    key = ("nc", False)
    if key not in _CACHE:
        _CACHE[key] = _build_module(uniform_d=False)
    nc = _CACHE[key]

    shared = {
        "alphaT": alphaT_dup,
        "stat3T": stat3T,
        "frhs": frhs,
        "dmask": dmask,
    }
    in_maps = [
        {"xs": x[i * BPC : (i + 1) * BPC], **shared} for i in range(NCORES)
    ]
    res = run_bass_kernel_spmd(nc, in_maps, core_ids=list(range(NCORES)))
    outs = [res.results[i]["out"] for i in range(NCORES)]
    return np.concatenate(outs, axis=0).astype(np.float32)


def kernel(x, alpha_1, alpha_2, alpha_3, discount):
    x = np.ascontiguousarray(np.asarray(x, dtype=np.float32))
    a1 = np.asarray(alpha_1, np.float32)
    a2 = np.asarray(alpha_2, np.float32)
    a3 = np.asarray(alpha_3, np.float32)
    ds = np.asarray(discount, dtype=np.float64).reshape(T)

    if np.all(ds == ds[0]):
        return kernel_v2(x, a1, a2, a3, float(ds[0]))
    return _kernel_v1(x, a1, a2, a3, discount)


if __name__ == "__main__":
    import reference as ref

    inputs = {k: np.asarray(v) for k, v in ref.setup_inputs().items()}
    got = kernel(**inputs)
    print("kernel output shape:", got.shape, got.dtype)
